# revision 13
# baseline (speedup 1.0000x reference)
"""Trainium2 Bass kernel for LuluAttention (gated GQA attention + RoPE).

Sharding over 8 NeuronCores: core = b*4 + g where b = batch (2), g = head
group (4). Each core computes 4 Q heads + their shared KV head for one batch
element, plus the matching gate slice, and a partial o_proj output
(contraction over its 512 attn dims). Host sums the 4 partials per batch.

All on-chip tensors are kept in transposed layout ([dim, seq]) so the
attention pipeline needs no on-chip transposes:
  qT/kT [d, s]  -> scoresT[sk, sq] = kT_tile.T @ qT_chunk
  softmax over sk (partition dim): denominator via ones-matmul, broadcast of
  the reciprocal via a K=1 matmul.
  v kept straight [s, d] -> attnT[d, sq] = v_tile.T @ probsT
  agT[d, sq] = attnT * recip * gateT  feeds o_proj directly as lhsT.
RoPE rotate-half needs a cross-partition rotation by 64: done with two DMA
copies, signs folded into the host-precomputed sin table.

Perf structure (v3):
  - Dummy matmuls at kernel start keep the PE busy while the first weight/x
    DMAs land, so HAM is warm when real work starts.
  - Weight DMAs issue on the sync queue in first-use order; the RoPE rotate
    DMAs go through the scalar (ACT) DGE queue so they never sit behind
    megabytes of weight traffic.
  - Attention processes heads in pairs: the two heads' score/av/denominator
    tiles interleave, hiding the exp (scalar engine) latency.
  - Softmax normalization (reciprocal -> broadcast matmul -> muls) for each
    head pair is deferred into the next pair / the o_proj stream so the PE
    never waits on the DVE reciprocal.
  - o_proj of chunk c is issued after attention of chunk c+1; its PSUM
    groups cycle a 4-deep ring shared with the projection and attention
    accumulators, and its output casts run on the otherwise-idle GpSimd.
  - Diagonal causal tiles only compute the live column range.
  - fp16 partial outputs (host accumulates in fp32), bf16 rope tables.
"""

import numpy as np
import ml_dtypes
from contextlib import ExitStack

import concourse.bass as bass
import concourse.bacc as bacc
import concourse.tile as tile
from concourse import mybir
from concourse.bass_utils import run_bass_kernel_spmd

BF16 = ml_dtypes.bfloat16

HIDDEN = 2048
B = 2
S_FULL = 2048
P = 128
CH = 512               # seq chunk width
QH = 4                 # q heads per core
DQ = QH * P            # 512 q dims per core
KT = HIDDEN // P       # 16 contraction tiles
SCALE = 1.0 / float(np.sqrt(128.0))
ROPE_THETA = 10000.0
WARMUP_MM = 60


def build_program(S=S_FULL):
    f32 = mybir.dt.float32
    f16 = mybir.dt.float16
    bf16 = mybir.dt.bfloat16
    sig = mybir.ActivationFunctionType.Sigmoid
    expf = mybir.ActivationFunctionType.Exp

    NCH = S // CH
    ST = CH // P           # 4 seq sub-tiles per chunk

    nc = bacc.Bacc("TRN2", debug=False, target_bir_lowering=False)

    xT = nc.declare_dram_parameter("xT", [HIDDEN, S], bf16, False)
    wq = nc.declare_dram_parameter("wq", [HIDDEN, DQ], bf16, False)
    wk = nc.declare_dram_parameter("wk", [HIDDEN, P], bf16, False)
    wv = nc.declare_dram_parameter("wv", [HIDDEN, P], bf16, False)
    wg = nc.declare_dram_parameter("wg", [HIDDEN, DQ], bf16, False)
    wo = nc.declare_dram_parameter("wo", [DQ, HIDDEN], bf16, False)
    bg = nc.declare_dram_parameter("bg", [DQ], f32, False)
    cosT = nc.declare_dram_parameter("cosT", [P, S], bf16, False)
    sinT = nc.declare_dram_parameter("sinT", [P, S], bf16, False)
    msk = nc.declare_dram_parameter("msk", [ST, P, CH], bf16, False)
    out = nc.declare_dram_parameter("out", [S, HIDDEN], f16, True)

    with tile.TileContext(nc) as tc, ExitStack() as ctx:
        wpool = ctx.enter_context(tc.tile_pool(name="weights", bufs=1))
        xpool = ctx.enter_context(tc.tile_pool(name="xchunks", bufs=2))
        qkv = ctx.enter_context(tc.tile_pool(name="qkv", bufs=1))
        work = ctx.enter_context(tc.tile_pool(name="work", bufs=4))
        agp = ctx.enter_context(tc.tile_pool(name="agp", bufs=2))
        outp = ctx.enter_context(tc.tile_pool(name="outp", bufs=4))
        # PSUM: 4 + 2 + 2 = 8 banks.
        ps_pj = ctx.enter_context(tc.tile_pool(name="ps_pj", bufs=4, space="PSUM"))
        ps_sc = ctx.enter_context(tc.tile_pool(name="ps_sc", bufs=2, space="PSUM"))
        ps_dn = ctx.enter_context(tc.tile_pool(name="ps_dn", bufs=2, space="PSUM"))

        # ---- persistent loads, ordered by first use (sync DGE queue) ----
        wq_sb = wpool.tile([P, KT, DQ], bf16, tag="wq")
        nc.sync.dma_start(out=wq_sb, in_=wq[:, :].rearrange("(kt p) n -> p kt n", p=P))

        xc_tiles = [None] * NCH

        def load_xc(c):
            t = xpool.tile([P, KT, CH], bf16, tag="xc", name=f"xc{c}")
            nc.sync.dma_start(
                out=t, in_=xT[:, c * CH:(c + 1) * CH].rearrange("(kt p) n -> p kt n", p=P)
            )
            xc_tiles[c] = t

        load_xc(0)

        wk_sb = wpool.tile([P, KT, P], bf16, tag="wk")
        nc.sync.dma_start(out=wk_sb, in_=wk[:, :].rearrange("(kt p) n -> p kt n", p=P))
        wv_sb = wpool.tile([P, KT, P], bf16, tag="wv")
        nc.sync.dma_start(out=wv_sb, in_=wv[:, :].rearrange("(kt p) n -> p kt n", p=P))
        cos_sb = wpool.tile([P, S], bf16, tag="cos")
        nc.sync.dma_start(out=cos_sb, in_=cosT[:, :])
        sin_sb = wpool.tile([P, S], bf16, tag="sin")
        nc.sync.dma_start(out=sin_sb, in_=sinT[:, :])
        msk_sb = wpool.tile([P, ST, CH], bf16, tag="msk")
        nc.sync.dma_start(out=msk_sb, in_=msk[:, :, :].rearrange("o p n -> p o n"))
        bg_sb = wpool.tile([P, QH], f32, tag="bg")
        nc.sync.dma_start(out=bg_sb, in_=bg[:].rearrange("(h p) -> p h", p=P))
        wg_sb = wpool.tile([P, KT, DQ], bf16, tag="wg")
        nc.sync.dma_start(out=wg_sb, in_=wg[:, :].rearrange("(kt p) n -> p kt n", p=P))
        wo_sb = wpool.tile([P, QH, HIDDEN], bf16, tag="wo")
        nc.sync.dma_start(out=wo_sb, in_=wo[:, :].rearrange("(dt p) n -> p dt n", p=P))

        ones_pv = wpool.tile([P, 1], bf16, tag="ones_pv")
        nc.vector.memset(ones_pv, 1.0)
        ones_bc = wpool.tile([1, P], f32, tag="ones_bc")
        nc.vector.memset(ones_bc, 1.0)

        # ---- HAM warmup: keep PE busy while the first DMAs land ----
        warm_in = wpool.tile([P, CH], bf16, tag="warm")
        nc.vector.memset(warm_in, 0.0)
        for i in range(WARMUP_MM):
            wps = ps_sc.tile([P, CH], f32, tag="sc", name=f"warm{i}")
            nc.tensor.matmul(wps, warm_in[:, 0:P], warm_in, start=True, stop=True)

        # persistent per-core activations (transposed layouts)
        qro = qkv.tile([P, QH, S], bf16, tag="qro")
        kro = qkv.tile([P, S], bf16, tag="kro")
        v_sb = qkv.tile([P, S // P, P], bf16, tag="v")
        gt = qkv.tile([P, QH, S], bf16, tag="gt")

        ag_prev = None  # (chunk_idx, ag_tile)

        for c in range(NCH):
            cs = slice(c * CH, (c + 1) * CH)
            xc = xc_tiles[c]
            if c + 1 < NCH:
                load_xc(c + 1)

            def proj_qk():
                # q heads + k, with RoPE applied out of PSUM
                for qh in range(QH + 1):
                    ps = ps_pj.tile([P, CH], f32, tag="proj")
                    for kt in range(KT):
                        lhs = (
                            wq_sb[:, kt, qh * P:(qh + 1) * P]
                            if qh < QH
                            else wk_sb[:, kt, :]
                        )
                        nc.tensor.matmul(
                            ps, lhs, xc[:, kt, :], start=(kt == 0), stop=(kt == KT - 1)
                        )
                    qf = work.tile([P, CH], f32, tag="qf")
                    nc.vector.tensor_copy(out=qf, in_=ps)
                    rot = work.tile([P, CH], f32, tag="rot")
                    nc.scalar.dma_start(out=rot[0:64, :], in_=qf[64:128, :])
                    nc.scalar.dma_start(out=rot[64:128, :], in_=qf[0:64, :])
                    t1 = work.tile([P, CH], f32, tag="t1")
                    nc.vector.tensor_mul(t1, qf, cos_sb[:, cs])
                    t2 = work.tile([P, CH], f32, tag="t2")
                    nc.vector.tensor_mul(t2, rot, sin_sb[:, cs])
                    dst = qro[:, qh, cs] if qh < QH else kro[:, cs]
                    nc.vector.tensor_add(dst, t1, t2)

            def proj_v():
                # v in straight layout [s, d]
                for st in range(ST):
                    s0 = c * ST + st
                    ps = ps_pj.tile([P, P], f32, tag="proj")
                    for kt in range(KT):
                        nc.tensor.matmul(
                            ps,
                            xc[:, kt, st * P:(st + 1) * P],
                            wv_sb[:, kt, :],
                            start=(kt == 0),
                            stop=(kt == KT - 1),
                        )
                    nc.vector.tensor_copy(out=v_sb[:, s0, :], in_=ps)

            def proj_gate():
                # gate heads: sigmoid(x @ Wg + bg), transposed layout
                for qh in range(QH):
                    ps = ps_pj.tile([P, CH], f32, tag="proj")
                    for kt in range(KT):
                        nc.tensor.matmul(
                            ps,
                            wg_sb[:, kt, qh * P:(qh + 1) * P],
                            xc[:, kt, :],
                            start=(kt == 0),
                            stop=(kt == KT - 1),
                        )
                    nc.scalar.activation(
                        out=gt[:, qh, cs],
                        in_=ps,
                        func=sig,
                        bias=bg_sb[:, qh:qh + 1],
                        scale=1.0,
                    )

            if c == 0:
                # wg is near the end of the weight-load queue: q/k first
                proj_qk(); proj_v(); proj_gate()
            else:
                # gate first: the sigmoid table swap overlaps gate matmuls
                # instead of blocking the q-proj PSUM recycle
                proj_gate(); proj_qk(); proj_v()

            # ---- attention for this sq chunk, heads in pairs ----
            ag = agp.tile([P, QH, CH], bf16, tag="ag")
            ntiles = (c + 1) * ST

            def norm_bc(rc, name):
                bc = ps_sc.tile([P, CH], f32, tag="sc", name=name)
                nc.tensor.matmul(bc, ones_bc, rc, start=True, stop=True)
                return bc

            def norm_t3(qh, at):
                # PSUM x SBUF (gate) first — frees the at bank early and
                # keeps both muls to a single PSUM operand each.
                t3 = work.tile([P, CH], f32, tag="t3", bufs=3)
                nc.vector.tensor_mul(t3, at, gt[:, qh, cs])
                return t3

            def norm_ag(qh, t3, bc):
                nc.vector.tensor_mul(ag[:, qh, :], t3, bc)

            def attn_tile(t, qh, at, dn, sc_name):
                o = t - c * ST
                off = o * P if o > 0 else 0  # live cols of diagonal tiles
                sc_ps = ps_sc.tile([P, CH], f32, tag="sc", name=sc_name)
                nc.tensor.matmul(
                    sc_ps[:, off:],
                    kro[:, t * P:(t + 1) * P],
                    qro[:, qh, c * CH + off:(c + 1) * CH],
                    start=True,
                    stop=True,
                )
                pr = work.tile([P, CH], bf16, tag="probs", bufs=4)
                nc.scalar.activation(
                    out=pr[:, off:], in_=sc_ps[:, off:], func=expf, scale=SCALE
                )
                if o >= 0:
                    nc.vector.tensor_mul(pr[:, off:], pr[:, off:], msk_sb[:, o, off:])
                return pr, off

            def attn_accum(t, pr, off, at, dn):
                nc.tensor.matmul(
                    at[:, off:], v_sb[:, t, :], pr[:, off:],
                    start=(t == 0), stop=(t == ntiles - 1),
                )
                nc.tensor.matmul(
                    dn[:, off:], ones_pv, pr[:, off:],
                    start=(t == 0), stop=(t == ntiles - 1),
                )

            pend = []  # [(qh, at, rc)] awaiting normalization
            for ha, hb in ((0, 1), (2, 3)):
                at_a = ps_pj.tile([P, CH], f32, tag="proj", name="at_a")
                at_b = ps_pj.tile([P, CH], f32, tag="proj", name="at_b")
                dn_a = ps_dn.tile([1, CH], f32, tag="dn", name="dn_a")
                dn_b = ps_dn.tile([1, CH], f32, tag="dn", name="dn_b")
                for t in range(ntiles):
                    pr_a, off = attn_tile(t, ha, at_a, dn_a, "sc_a")
                    pr_b, _ = attn_tile(t, hb, at_b, dn_b, "sc_b")
                    # normalize the previous pair inside the exp-latency
                    # bubble of this pair's first two tiles
                    if t < 2 and pend:
                        qh_p, at_p, rc_p = pend.pop(0)
                        t3_p = norm_t3(qh_p, at_p)
                        norm_ag(qh_p, t3_p, norm_bc(rc_p, f"bc{qh_p}"))
                    attn_accum(t, pr_a, off, at_a, dn_a)
                    attn_accum(t, pr_b, off, at_b, dn_b)
                rc_a = work.tile([1, CH], f32, tag="recip", bufs=4)
                nc.vector.reciprocal_approx_fast(out=rc_a, in_=dn_a)
                rc_b = work.tile([1, CH], f32, tag="recip", bufs=4)
                nc.vector.reciprocal_approx_fast(out=rc_b, in_=dn_b)
                pend += [(ha, at_a, rc_a), (hb, at_b, rc_b)]

            # o_proj of the previous chunk, with the last pair's
            # normalization woven into the first few groups.
            self_norm = pend
            pend = []
            if ag_prev is not None:
                emit_oproj(nc, ps_pj, outp, out, wo_sb, ag_prev, self_norm,
                           norm_bc, norm_t3, norm_ag)
            else:
                t3s = [norm_t3(qh, at) for qh, at, _ in self_norm]
                bcs = [norm_bc(rc, f"bc{qh}") for qh, _, rc in self_norm]
                for (qh, _, _), t3, bc in zip(self_norm, t3s, bcs):
                    norm_ag(qh, t3, bc)
            ag_prev = (c, ag)

        emit_oproj(nc, ps_pj, outp, out, wo_sb, ag_prev, [], None, None, None)

    nc.finalize()
    return nc


def emit_oproj(nc, ps_pj, outp, out, wo_sb, ag_info, norm2,
               norm_bc, norm_t3, norm_ag):
    f32 = mybir.dt.float32
    f16 = mybir.dt.float16
    c, ag = ag_info
    ST = CH // P
    bcs = []
    t3s = []
    gi = 0
    for st in range(ST):
        r0 = c * CH + st * P
        for h0 in range(HIDDEN // CH):
            if gi == 0 and norm2:
                # t3 muls free the at banks that groups 2/3 will reuse;
                # they only need the gate, so they run during group 0/1
                t3s = [norm_t3(qh, at) for qh, at, _ in norm2]
            if gi == 2 and norm2:
                # bc matmuls for the last pair (reciprocals are long done)
                bcs = [norm_bc(rc, f"bc{qh}") for qh, _, rc in norm2]
            if gi == 4 and norm2:
                for (qh, _, _), t3, bc in zip(norm2, t3s, bcs):
                    norm_ag(qh, t3, bc)
            ps = ps_pj.tile([P, CH], f32, tag="proj", name=f"op{gi}")
            for dt in range(QH):
                nc.tensor.matmul(
                    ps,
                    ag[:, dt, st * P:(st + 1) * P],
                    wo_sb[:, dt, h0 * CH:(h0 + 1) * CH],
                    start=(dt == 0),
                    stop=(dt == QH - 1),
                )
            ob = outp.tile([P, CH], f16, tag="ob")
            nc.vector.tensor_copy(out=ob, in_=ps)
            nc.sync.dma_start(out=out[r0:r0 + P, h0 * CH:(h0 + 1) * CH], in_=ob)
            gi += 1


_PROGRAMS = {}


def _get_program(S=S_FULL):
    if S not in _PROGRAMS:
        _PROGRAMS[S] = build_program(S)
    return _PROGRAMS[S]


def _host_tables(position_ids_b, S):
    pos = np.asarray(position_ids_b, dtype=np.float32)  # [S]
    inv = 1.0 / (ROPE_THETA ** (np.arange(0, P, 2, dtype=np.float32) / P))  # [64]
    ang = np.concatenate([inv, inv]).astype(np.float32)[:, None] * pos[None, :]
    cosT = np.cos(ang).astype(BF16)
    sgn = np.where(np.arange(P) < 64, -1.0, 1.0).astype(np.float32)
    sinT = (np.sin(ang) * sgn[:, None]).astype(BF16)
    return cosT, sinT


def _causal_masks():
    o = np.arange(CH // P)[:, None, None]
    r = np.arange(P)[None, :, None]
    j = np.arange(CH)[None, None, :]
    return ((P * o + r) <= j).astype(BF16)


def make_in_maps(x, position_ids, Wq, Wk, Wv, Wo, Wg, bg, S=S_FULL):
    x = np.asarray(x, dtype=np.float32)
    msk = _causal_masks()
    maps = []
    xT_b = [np.ascontiguousarray(x[b, :S].T).astype(BF16) for b in range(B)]
    tabs = [_host_tables(np.asarray(position_ids)[b, :S], S) for b in range(B)]
    Wq = np.asarray(Wq, np.float32)
    Wk = np.asarray(Wk, np.float32)
    Wv = np.asarray(Wv, np.float32)
    Wo = np.asarray(Wo, np.float32)
    Wg = np.asarray(Wg, np.float32)
    bg = np.asarray(bg, np.float32)
    for core in range(8):
        b, g = core // 4, core % 4
        cosT, sinT = tabs[b]
        maps.append({
            "xT": xT_b[b],
            "wq": np.ascontiguousarray(Wq[:, g * DQ:(g + 1) * DQ]).astype(BF16),
            "wk": np.ascontiguousarray(Wk[:, g * P:(g + 1) * P]).astype(BF16),
            "wv": np.ascontiguousarray(Wv[:, g * P:(g + 1) * P]).astype(BF16),
            "wg": np.ascontiguousarray(Wg[:, g * DQ:(g + 1) * DQ]).astype(BF16),
            "wo": np.ascontiguousarray(Wo[g * DQ:(g + 1) * DQ, :]).astype(BF16),
            "bg": np.ascontiguousarray(bg[g * DQ:(g + 1) * DQ]),
            "cosT": cosT,
            "sinT": sinT,
            "msk": msk,
        })
    return maps


def run(inputs, S=S_FULL, trace=False, **kw):
    nc = _get_program(S)
    maps = make_in_maps(S=S, **inputs)
    res = run_bass_kernel_spmd(nc, maps, core_ids=list(range(8)), trace=trace, **kw)
    out = np.zeros((B, S, HIDDEN), np.float32)
    for core in range(8):
        out[core // 4] += np.asarray(res.results[core]["out"], np.float32)
    return out, res


def kernel(x, position_ids, Wq, Wk, Wv, Wo, Wg, bg):
    out, _ = run(dict(x=x, position_ids=position_ids, Wq=Wq, Wk=Wk, Wv=Wv,
                      Wo=Wo, Wg=Wg, bg=bg))
    return out


# revision 14
# speedup vs baseline: 1.0178x; 1.0178x over previous
"""Trainium2 Bass kernel for LuluAttention (gated GQA attention + RoPE).

Sharding over 8 NeuronCores: core = b*4 + g where b = batch (2), g = head
group (4). Each core computes 4 Q heads + their shared KV head for one batch
element, plus the matching gate slice, and a partial o_proj output
(contraction over its 512 attn dims). Host sums the 4 partials per batch.

All on-chip tensors are kept in transposed layout ([dim, seq]) so the
attention pipeline needs no on-chip transposes:
  qT/kT [d, s]  -> scoresT[sk, sq] = kT_tile.T @ qT_chunk
  softmax over sk (partition dim): denominator via ones-matmul, broadcast of
  the reciprocal via a K=1 matmul.
  v kept straight [s, d] -> attnT[d, sq] = v_tile.T @ probsT
  agT[d, sq] = attnT * recip * gateT  feeds o_proj directly as lhsT.
RoPE rotate-half needs a cross-partition rotation by 64: done with two DMA
copies, signs folded into the host-precomputed sin table.

Perf structure (v3):
  - Dummy matmuls at kernel start keep the PE busy while the first weight/x
    DMAs land, so HAM is warm when real work starts.
  - Weight DMAs issue on the sync queue in first-use order; the RoPE rotate
    DMAs go through the scalar (ACT) DGE queue so they never sit behind
    megabytes of weight traffic.
  - Attention processes heads in pairs: the two heads' score/av/denominator
    tiles interleave, hiding the exp (scalar engine) latency.
  - Softmax normalization (reciprocal -> broadcast matmul -> muls) for each
    head pair is deferred into the next pair / the o_proj stream so the PE
    never waits on the DVE reciprocal.
  - o_proj of chunk c is issued after attention of chunk c+1; its PSUM
    groups cycle a 4-deep ring shared with the projection and attention
    accumulators, and its output casts run on the otherwise-idle GpSimd.
  - Diagonal causal tiles only compute the live column range.
  - fp16 partial outputs (host accumulates in fp32), bf16 rope tables.
"""

import numpy as np
import ml_dtypes
from contextlib import ExitStack

import concourse.bass as bass
import concourse.bacc as bacc
import concourse.tile as tile
from concourse import mybir
from concourse.bass_utils import run_bass_kernel_spmd

BF16 = ml_dtypes.bfloat16

HIDDEN = 2048
B = 2
S_FULL = 2048
P = 128
CH = 512               # seq chunk width
QH = 4                 # q heads per core
DQ = QH * P            # 512 q dims per core
KT = HIDDEN // P       # 16 contraction tiles
SCALE = 1.0 / float(np.sqrt(128.0))
ROPE_THETA = 10000.0
WARMUP_MM = 60


def build_program(S=S_FULL):
    f32 = mybir.dt.float32
    f16 = mybir.dt.float16
    bf16 = mybir.dt.bfloat16
    tanh = mybir.ActivationFunctionType.Tanh
    expf = mybir.ActivationFunctionType.Exp

    NCH = S // CH
    ST = CH // P           # 4 seq sub-tiles per chunk

    nc = bacc.Bacc("TRN2", debug=False, target_bir_lowering=False)

    xT = nc.declare_dram_parameter("xT", [HIDDEN, S], bf16, False)
    wq = nc.declare_dram_parameter("wq", [HIDDEN, DQ], bf16, False)
    wk = nc.declare_dram_parameter("wk", [HIDDEN, P], bf16, False)
    wv = nc.declare_dram_parameter("wv", [HIDDEN, P], bf16, False)
    wg = nc.declare_dram_parameter("wg", [HIDDEN, DQ], bf16, False)
    wo = nc.declare_dram_parameter("wo", [DQ, HIDDEN], bf16, False)
    bg = nc.declare_dram_parameter("bg", [DQ], f32, False)
    cosT = nc.declare_dram_parameter("cosT", [P, S], bf16, False)
    sinT = nc.declare_dram_parameter("sinT", [P, S], bf16, False)
    msk = nc.declare_dram_parameter("msk", [ST, P, CH], bf16, False)
    out = nc.declare_dram_parameter("out", [S, HIDDEN], f16, True)

    with tile.TileContext(nc) as tc, ExitStack() as ctx:
        wpool = ctx.enter_context(tc.tile_pool(name="weights", bufs=1))
        xpool = ctx.enter_context(tc.tile_pool(name="xchunks", bufs=2))
        qkv = ctx.enter_context(tc.tile_pool(name="qkv", bufs=1))
        work = ctx.enter_context(tc.tile_pool(name="work", bufs=4))
        agp = ctx.enter_context(tc.tile_pool(name="agp", bufs=2))
        outp = ctx.enter_context(tc.tile_pool(name="outp", bufs=4))
        # PSUM: 4 + 2 + 2 = 8 banks.
        ps_pj = ctx.enter_context(tc.tile_pool(name="ps_pj", bufs=4, space="PSUM"))
        ps_sc = ctx.enter_context(tc.tile_pool(name="ps_sc", bufs=2, space="PSUM"))
        ps_dn = ctx.enter_context(tc.tile_pool(name="ps_dn", bufs=2, space="PSUM"))

        # ---- persistent loads, ordered by first use (sync DGE queue) ----
        wq_sb = wpool.tile([P, KT, DQ], bf16, tag="wq")
        nc.sync.dma_start(out=wq_sb, in_=wq[:, :].rearrange("(kt p) n -> p kt n", p=P))

        xc_tiles = [None] * NCH

        def load_xc(c):
            t = xpool.tile([P, KT, CH], bf16, tag="xc", name=f"xc{c}")
            nc.sync.dma_start(
                out=t, in_=xT[:, c * CH:(c + 1) * CH].rearrange("(kt p) n -> p kt n", p=P)
            )
            xc_tiles[c] = t

        load_xc(0)

        wk_sb = wpool.tile([P, KT, P], bf16, tag="wk")
        nc.sync.dma_start(out=wk_sb, in_=wk[:, :].rearrange("(kt p) n -> p kt n", p=P))
        wv_sb = wpool.tile([P, KT, P], bf16, tag="wv")
        nc.sync.dma_start(out=wv_sb, in_=wv[:, :].rearrange("(kt p) n -> p kt n", p=P))
        cos_sb = wpool.tile([P, S], bf16, tag="cos")
        nc.sync.dma_start(out=cos_sb, in_=cosT[:, :])
        sin_sb = wpool.tile([P, S], bf16, tag="sin")
        nc.sync.dma_start(out=sin_sb, in_=sinT[:, :])
        msk_sb = wpool.tile([P, ST, CH], bf16, tag="msk")
        nc.sync.dma_start(out=msk_sb, in_=msk[:, :, :].rearrange("o p n -> p o n"))
        bg_sb = wpool.tile([P, QH], f32, tag="bg")
        nc.sync.dma_start(out=bg_sb, in_=bg[:].rearrange("(h p) -> p h", p=P))
        wg_sb = wpool.tile([P, KT, DQ], bf16, tag="wg")
        nc.sync.dma_start(out=wg_sb, in_=wg[:, :].rearrange("(kt p) n -> p kt n", p=P))
        wo_sb = wpool.tile([P, QH, HIDDEN], bf16, tag="wo")
        nc.sync.dma_start(out=wo_sb, in_=wo[:, :].rearrange("(dt p) n -> p dt n", p=P))

        ones_pv = wpool.tile([P, 1], bf16, tag="ones_pv")
        nc.vector.memset(ones_pv, 1.0)
        ones_bc = wpool.tile([1, P], f32, tag="ones_bc")
        nc.vector.memset(ones_bc, 1.0)

        # ---- HAM warmup: keep PE busy while the first DMAs land ----
        warm_in = wpool.tile([P, CH], bf16, tag="warm")
        nc.vector.memset(warm_in, 0.0)
        for i in range(WARMUP_MM):
            wps = ps_sc.tile([P, CH], f32, tag="sc", name=f"warm{i}")
            nc.tensor.matmul(wps, warm_in[:, 0:P], warm_in, start=True, stop=True)

        # persistent per-core activations (transposed layouts)
        qro = qkv.tile([P, QH, S], bf16, tag="qro")
        kro = qkv.tile([P, S], bf16, tag="kro")
        v_sb = qkv.tile([P, S // P, P], bf16, tag="v")
        gt = qkv.tile([P, QH, S], bf16, tag="gt")

        ag_prev = None  # (chunk_idx, ag_tile)

        for c in range(NCH):
            cs = slice(c * CH, (c + 1) * CH)
            xc = xc_tiles[c]
            if c + 1 < NCH:
                load_xc(c + 1)

            def proj_qk():
                # q heads + k, with RoPE applied out of PSUM
                for qh in range(QH + 1):
                    ps = ps_pj.tile([P, CH], f32, tag="proj")
                    for kt in range(KT):
                        lhs = (
                            wq_sb[:, kt, qh * P:(qh + 1) * P]
                            if qh < QH
                            else wk_sb[:, kt, :]
                        )
                        nc.tensor.matmul(
                            ps, lhs, xc[:, kt, :], start=(kt == 0), stop=(kt == KT - 1)
                        )
                    qf = work.tile([P, CH], f32, tag="qf")
                    nc.scalar.copy(out=qf, in_=ps)
                    rot = work.tile([P, CH], f32, tag="rot")
                    nc.scalar.dma_start(out=rot[0:64, :], in_=qf[64:128, :])
                    nc.scalar.dma_start(out=rot[64:128, :], in_=qf[0:64, :])
                    t1 = work.tile([P, CH], f32, tag="t1")
                    nc.vector.tensor_mul(t1, qf, cos_sb[:, cs])
                    t2 = work.tile([P, CH], f32, tag="t2")
                    nc.vector.tensor_mul(t2, rot, sin_sb[:, cs])
                    dst = qro[:, qh, cs] if qh < QH else kro[:, cs]
                    nc.vector.tensor_add(dst, t1, t2)

            def proj_v():
                # v in straight layout [s, d]
                for st in range(ST):
                    s0 = c * ST + st
                    ps = ps_pj.tile([P, P], f32, tag="proj")
                    for kt in range(KT):
                        nc.tensor.matmul(
                            ps,
                            xc[:, kt, st * P:(st + 1) * P],
                            wv_sb[:, kt, :],
                            start=(kt == 0),
                            stop=(kt == KT - 1),
                        )
                    nc.scalar.copy(out=v_sb[:, s0, :], in_=ps)

            def proj_gate():
                # gate heads: sigmoid(x @ Wg + bg), transposed layout
                for qh in range(QH):
                    ps = ps_pj.tile([P, CH], f32, tag="proj")
                    for kt in range(KT):
                        nc.tensor.matmul(
                            ps,
                            wg_sb[:, kt, qh * P:(qh + 1) * P],
                            xc[:, kt, :],
                            start=(kt == 0),
                            stop=(kt == KT - 1),
                        )
                    th = work.tile([P, CH], bf16, tag="gth", bufs=2)
                    nc.scalar.activation(
                        out=th,
                        in_=ps,
                        func=tanh,
                        bias=bg_sb[:, qh:qh + 1],
                        scale=0.5,
                    )
                    nc.vector.tensor_scalar(
                        out=gt[:, qh, cs], in0=th,
                        scalar1=0.5, scalar2=0.5,
                        op0=mybir.AluOpType.mult, op1=mybir.AluOpType.add,
                    )

            if c == 0:
                # wg is near the end of the weight-load queue: q/k first
                proj_qk(); proj_v(); proj_gate()
            else:
                # gate first: the sigmoid table swap overlaps gate matmuls
                # instead of blocking the q-proj PSUM recycle
                proj_gate(); proj_qk(); proj_v()

            # ---- attention for this sq chunk, heads in pairs ----
            ag = agp.tile([P, QH, CH], bf16, tag="ag")
            ntiles = (c + 1) * ST

            def norm_bc(rc, name):
                bc = ps_sc.tile([P, CH], f32, tag="sc", name=name)
                nc.tensor.matmul(bc, ones_bc, rc, start=True, stop=True)
                return bc

            def norm_t3(qh, at):
                # PSUM x SBUF (gate) first — frees the at bank early and
                # keeps both muls to a single PSUM operand each.
                t3 = work.tile([P, CH], f32, tag="t3", bufs=3)
                nc.vector.tensor_mul(t3, at, gt[:, qh, cs])
                return t3

            def norm_ag(qh, t3, bc):
                nc.vector.tensor_mul(ag[:, qh, :], t3, bc)

            def attn_tile(t, qh, at, dn, sc_name):
                o = t - c * ST
                off = o * P if o > 0 else 0  # live cols of diagonal tiles
                sc_ps = ps_sc.tile([P, CH], f32, tag="sc", name=sc_name)
                nc.tensor.matmul(
                    sc_ps[:, off:],
                    kro[:, t * P:(t + 1) * P],
                    qro[:, qh, c * CH + off:(c + 1) * CH],
                    start=True,
                    stop=True,
                )
                pr = work.tile([P, CH], bf16, tag="probs", bufs=4)
                nc.scalar.activation(
                    out=pr[:, off:], in_=sc_ps[:, off:], func=expf, scale=SCALE
                )
                if o >= 0:
                    nc.vector.tensor_mul(pr[:, off:], pr[:, off:], msk_sb[:, o, off:])
                return pr, off

            def attn_accum(t, pr, off, at, dn):
                nc.tensor.matmul(
                    at[:, off:], v_sb[:, t, :], pr[:, off:],
                    start=(t == 0), stop=(t == ntiles - 1),
                )
                nc.tensor.matmul(
                    dn[:, off:], ones_pv, pr[:, off:],
                    start=(t == 0), stop=(t == ntiles - 1),
                )

            pend = []  # [(qh, at, rc)] awaiting normalization
            for ha, hb in ((0, 1), (2, 3)):
                at_a = ps_pj.tile([P, CH], f32, tag="proj", name="at_a")
                at_b = ps_pj.tile([P, CH], f32, tag="proj", name="at_b")
                dn_a = ps_dn.tile([1, CH], f32, tag="dn", name="dn_a")
                dn_b = ps_dn.tile([1, CH], f32, tag="dn", name="dn_b")
                for t in range(ntiles):
                    pr_a, off = attn_tile(t, ha, at_a, dn_a, "sc_a")
                    pr_b, _ = attn_tile(t, hb, at_b, dn_b, "sc_b")
                    # normalize the previous pair inside the exp-latency
                    # bubble of this pair's first two tiles
                    if t < 2 and pend:
                        qh_p, at_p, rc_p = pend.pop(0)
                        t3_p = norm_t3(qh_p, at_p)
                        norm_ag(qh_p, t3_p, norm_bc(rc_p, f"bc{qh_p}"))
                    attn_accum(t, pr_a, off, at_a, dn_a)
                    attn_accum(t, pr_b, off, at_b, dn_b)
                rc_a = work.tile([1, CH], f32, tag="recip", bufs=4)
                nc.vector.reciprocal_approx_fast(out=rc_a, in_=dn_a)
                rc_b = work.tile([1, CH], f32, tag="recip", bufs=4)
                nc.vector.reciprocal_approx_fast(out=rc_b, in_=dn_b)
                pend += [(ha, at_a, rc_a), (hb, at_b, rc_b)]

            # o_proj of the previous chunk, with the last pair's
            # normalization woven into the first few groups.
            self_norm = pend
            pend = []
            if ag_prev is not None:
                emit_oproj(nc, ps_pj, outp, out, wo_sb, ag_prev, self_norm,
                           norm_bc, norm_t3, norm_ag)
            else:
                t3s = [norm_t3(qh, at) for qh, at, _ in self_norm]
                bcs = [norm_bc(rc, f"bc{qh}") for qh, _, rc in self_norm]
                for (qh, _, _), t3, bc in zip(self_norm, t3s, bcs):
                    norm_ag(qh, t3, bc)
            ag_prev = (c, ag)

        emit_oproj(nc, ps_pj, outp, out, wo_sb, ag_prev, [], None, None, None)

    nc.finalize()
    return nc


def emit_oproj(nc, ps_pj, outp, out, wo_sb, ag_info, norm2,
               norm_bc, norm_t3, norm_ag):
    f32 = mybir.dt.float32
    f16 = mybir.dt.float16
    c, ag = ag_info
    ST = CH // P
    bcs = []
    t3s = []
    gi = 0
    for st in range(ST):
        r0 = c * CH + st * P
        for h0 in range(HIDDEN // CH):
            if gi == 0 and norm2:
                # t3 muls free the at banks that groups 2/3 will reuse;
                # they only need the gate, so they run during group 0/1
                t3s = [norm_t3(qh, at) for qh, at, _ in norm2]
            if gi == 2 and norm2:
                # bc matmuls for the last pair (reciprocals are long done)
                bcs = [norm_bc(rc, f"bc{qh}") for qh, _, rc in norm2]
            if gi == 4 and norm2:
                for (qh, _, _), t3, bc in zip(norm2, t3s, bcs):
                    norm_ag(qh, t3, bc)
            ps = ps_pj.tile([P, CH], f32, tag="proj", name=f"op{gi}")
            for dt in range(QH):
                nc.tensor.matmul(
                    ps,
                    ag[:, dt, st * P:(st + 1) * P],
                    wo_sb[:, dt, h0 * CH:(h0 + 1) * CH],
                    start=(dt == 0),
                    stop=(dt == QH - 1),
                )
            ob = outp.tile([P, CH], f16, tag="ob")
            nc.vector.tensor_copy(out=ob, in_=ps)
            nc.sync.dma_start(out=out[r0:r0 + P, h0 * CH:(h0 + 1) * CH], in_=ob)
            gi += 1


_PROGRAMS = {}


def _get_program(S=S_FULL):
    if S not in _PROGRAMS:
        _PROGRAMS[S] = build_program(S)
    return _PROGRAMS[S]


def _host_tables(position_ids_b, S):
    pos = np.asarray(position_ids_b, dtype=np.float32)  # [S]
    inv = 1.0 / (ROPE_THETA ** (np.arange(0, P, 2, dtype=np.float32) / P))  # [64]
    ang = np.concatenate([inv, inv]).astype(np.float32)[:, None] * pos[None, :]
    cosT = np.cos(ang).astype(BF16)
    sgn = np.where(np.arange(P) < 64, -1.0, 1.0).astype(np.float32)
    sinT = (np.sin(ang) * sgn[:, None]).astype(BF16)
    return cosT, sinT


def _causal_masks():
    o = np.arange(CH // P)[:, None, None]
    r = np.arange(P)[None, :, None]
    j = np.arange(CH)[None, None, :]
    return ((P * o + r) <= j).astype(BF16)


def make_in_maps(x, position_ids, Wq, Wk, Wv, Wo, Wg, bg, S=S_FULL):
    x = np.asarray(x, dtype=np.float32)
    msk = _causal_masks()
    maps = []
    xT_b = [np.ascontiguousarray(x[b, :S].T).astype(BF16) for b in range(B)]
    tabs = [_host_tables(np.asarray(position_ids)[b, :S], S) for b in range(B)]
    Wq = np.asarray(Wq, np.float32)
    Wk = np.asarray(Wk, np.float32)
    Wv = np.asarray(Wv, np.float32)
    Wo = np.asarray(Wo, np.float32)
    Wg = np.asarray(Wg, np.float32)
    bg = np.asarray(bg, np.float32)
    for core in range(8):
        b, g = core // 4, core % 4
        cosT, sinT = tabs[b]
        maps.append({
            "xT": xT_b[b],
            "wq": np.ascontiguousarray(Wq[:, g * DQ:(g + 1) * DQ]).astype(BF16),
            "wk": np.ascontiguousarray(Wk[:, g * P:(g + 1) * P]).astype(BF16),
            "wv": np.ascontiguousarray(Wv[:, g * P:(g + 1) * P]).astype(BF16),
            "wg": np.ascontiguousarray(Wg[:, g * DQ:(g + 1) * DQ]).astype(BF16),
            "wo": np.ascontiguousarray(Wo[g * DQ:(g + 1) * DQ, :]).astype(BF16),
            "bg": np.ascontiguousarray(bg[g * DQ:(g + 1) * DQ]) * np.float32(0.5),
            "cosT": cosT,
            "sinT": sinT,
            "msk": msk,
        })
    return maps


def run(inputs, S=S_FULL, trace=False, **kw):
    nc = _get_program(S)
    maps = make_in_maps(S=S, **inputs)
    res = run_bass_kernel_spmd(nc, maps, core_ids=list(range(8)), trace=trace, **kw)
    out = np.zeros((B, S, HIDDEN), np.float32)
    for core in range(8):
        out[core // 4] += np.asarray(res.results[core]["out"], np.float32)
    return out, res


def kernel(x, position_ids, Wq, Wk, Wv, Wo, Wg, bg):
    out, _ = run(dict(x=x, position_ids=position_ids, Wq=Wq, Wk=Wk, Wv=Wv,
                      Wo=Wo, Wg=Wg, bg=bg))
    return out


# revision 15
# speedup vs baseline: 1.0524x; 1.0340x over previous
"""Trainium2 Bass kernel for LuluAttention (gated GQA attention + RoPE).

Sharding over 8 NeuronCores: core = b*4 + g where b = batch (2), g = head
group (4). Each core computes 4 Q heads + their shared KV head for one batch
element, plus the matching gate slice, and a partial o_proj output
(contraction over its 512 attn dims). Host sums the 4 partials per batch.

All on-chip tensors are kept in transposed layout ([dim, seq]) so the
attention pipeline needs no on-chip transposes:
  qT/kT [d, s]  -> scoresT[sk, sq] = kT_tile.T @ qT_chunk
  softmax over sk (partition dim): denominator via ones-matmul, broadcast of
  the reciprocal via a K=1 matmul.
  v kept straight [s, d] -> attnT[d, sq] = v_tile.T @ probsT
  agT[d, sq] = attnT * recip * gateT  feeds o_proj directly as lhsT.
RoPE rotate-half needs a cross-partition rotation by 64: done with two DMA
copies, signs folded into the host-precomputed sin table.

Perf structure (v3):
  - Dummy matmuls at kernel start keep the PE busy while the first weight/x
    DMAs land, so HAM is warm when real work starts.
  - Weight DMAs issue on the sync queue in first-use order; the RoPE rotate
    DMAs go through the scalar (ACT) DGE queue so they never sit behind
    megabytes of weight traffic.
  - Attention processes heads in pairs: the two heads' score/av/denominator
    tiles interleave, hiding the exp (scalar engine) latency.
  - Softmax normalization (reciprocal -> broadcast matmul -> muls) for each
    head pair is deferred into the next pair / the o_proj stream so the PE
    never waits on the DVE reciprocal.
  - o_proj of chunk c is issued after attention of chunk c+1; its PSUM
    groups cycle a 4-deep ring shared with the projection and attention
    accumulators, and its output casts run on the otherwise-idle GpSimd.
  - Diagonal causal tiles only compute the live column range.
  - fp16 partial outputs (host accumulates in fp32), bf16 rope tables.
"""

import numpy as np
import ml_dtypes
from contextlib import ExitStack

import concourse.bass as bass
import concourse.bacc as bacc
import concourse.tile as tile
from concourse import mybir
from concourse.bass_utils import run_bass_kernel_spmd

BF16 = ml_dtypes.bfloat16

HIDDEN = 2048
B = 2
S_FULL = 2048
P = 128
CH = 512               # seq chunk width
QH = 4                 # q heads per core
DQ = QH * P            # 512 q dims per core
KT = HIDDEN // P       # 16 contraction tiles
SCALE = 1.0 / float(np.sqrt(128.0))
ROPE_THETA = 10000.0
WARMUP_MM = 60


def build_program(S=S_FULL):
    f32 = mybir.dt.float32
    f16 = mybir.dt.float16
    bf16 = mybir.dt.bfloat16
    tanh = mybir.ActivationFunctionType.Tanh
    expf = mybir.ActivationFunctionType.Exp

    NCH = S // CH
    ST = CH // P           # 4 seq sub-tiles per chunk

    nc = bacc.Bacc("TRN2", debug=False, target_bir_lowering=False)

    xT = nc.declare_dram_parameter("xT", [HIDDEN, S], bf16, False)
    wq = nc.declare_dram_parameter("wq", [HIDDEN, DQ], bf16, False)
    wk = nc.declare_dram_parameter("wk", [HIDDEN, P], bf16, False)
    wv = nc.declare_dram_parameter("wv", [HIDDEN, P], bf16, False)
    wg = nc.declare_dram_parameter("wg", [HIDDEN, DQ], bf16, False)
    wo = nc.declare_dram_parameter("wo", [DQ, HIDDEN], bf16, False)
    bg = nc.declare_dram_parameter("bg", [DQ], f32, False)
    cosT = nc.declare_dram_parameter("cosT", [P, S], bf16, False)
    sinT = nc.declare_dram_parameter("sinT", [P, S], bf16, False)
    msk = nc.declare_dram_parameter("msk", [ST, P, CH], bf16, False)
    out = nc.declare_dram_parameter("out", [S, HIDDEN], f16, True)

    with tile.TileContext(nc) as tc, ExitStack() as ctx:
        wpool = ctx.enter_context(tc.tile_pool(name="weights", bufs=1))
        xpool = ctx.enter_context(tc.tile_pool(name="xchunks", bufs=2))
        qkv = ctx.enter_context(tc.tile_pool(name="qkv", bufs=1))
        work = ctx.enter_context(tc.tile_pool(name="work", bufs=4))
        agp = ctx.enter_context(tc.tile_pool(name="agp", bufs=2))
        outp = ctx.enter_context(tc.tile_pool(name="outp", bufs=4))
        # PSUM: 4 + 2 + 2 = 8 banks.
        ps_pj = ctx.enter_context(tc.tile_pool(name="ps_pj", bufs=4, space="PSUM"))
        ps_sc = ctx.enter_context(tc.tile_pool(name="ps_sc", bufs=2, space="PSUM"))
        ps_dn = ctx.enter_context(tc.tile_pool(name="ps_dn", bufs=2, space="PSUM"))

        # ---- persistent loads, ordered by first use (sync DGE queue) ----
        wq_sb = wpool.tile([P, KT, DQ], bf16, tag="wq")
        nc.sync.dma_start(out=wq_sb, in_=wq[:, :].rearrange("(kt p) n -> p kt n", p=P))

        xc_tiles = [None] * NCH

        def load_xc(c):
            t = xpool.tile([P, KT, CH], bf16, tag="xc", name=f"xc{c}")
            nc.sync.dma_start(
                out=t, in_=xT[:, c * CH:(c + 1) * CH].rearrange("(kt p) n -> p kt n", p=P)
            )
            xc_tiles[c] = t

        load_xc(0)

        wk_sb = wpool.tile([P, KT, P], bf16, tag="wk")
        nc.sync.dma_start(out=wk_sb, in_=wk[:, :].rearrange("(kt p) n -> p kt n", p=P))
        wv_sb = wpool.tile([P, KT, P], bf16, tag="wv")
        nc.sync.dma_start(out=wv_sb, in_=wv[:, :].rearrange("(kt p) n -> p kt n", p=P))
        cos_sb = wpool.tile([P, S], bf16, tag="cos")
        nc.sync.dma_start(out=cos_sb, in_=cosT[:, :])
        sin_sb = wpool.tile([P, S], bf16, tag="sin")
        nc.sync.dma_start(out=sin_sb, in_=sinT[:, :])
        msk_sb = wpool.tile([P, ST, CH], bf16, tag="msk")
        nc.sync.dma_start(out=msk_sb, in_=msk[:, :, :].rearrange("o p n -> p o n"))
        bg_sb = wpool.tile([P, QH], f32, tag="bg")
        nc.sync.dma_start(out=bg_sb, in_=bg[:].rearrange("(h p) -> p h", p=P))
        wg_sb = wpool.tile([P, KT, DQ], bf16, tag="wg")
        nc.sync.dma_start(out=wg_sb, in_=wg[:, :].rearrange("(kt p) n -> p kt n", p=P))
        wo_sb = wpool.tile([P, QH, HIDDEN], bf16, tag="wo")
        nc.sync.dma_start(out=wo_sb, in_=wo[:, :].rearrange("(dt p) n -> p dt n", p=P))

        ones_pv = wpool.tile([P, 1], bf16, tag="ones_pv")
        nc.vector.memset(ones_pv, 1.0)
        ones_bc = wpool.tile([1, P], f32, tag="ones_bc")
        nc.vector.memset(ones_bc, 1.0)

        # ---- HAM warmup: keep PE busy while the first DMAs land ----
        warm_in = wpool.tile([P, CH], bf16, tag="warm")
        nc.vector.memset(warm_in, 0.0)
        for i in range(WARMUP_MM):
            wps = ps_sc.tile([P, CH], f32, tag="sc", name=f"warm{i}")
            nc.tensor.matmul(wps, warm_in[:, 0:P], warm_in, start=True, stop=True)

        # persistent per-core activations (transposed layouts)
        qro = qkv.tile([P, QH, S], bf16, tag="qro")
        kro = qkv.tile([P, S], bf16, tag="kro")
        v_sb = qkv.tile([P, S // P, P], bf16, tag="v")
        gt = qkv.tile([P, QH, S], bf16, tag="gt")

        ag_prev = None  # (chunk_idx, ag_tile)

        for c in range(NCH):
            cs = slice(c * CH, (c + 1) * CH)
            xc = xc_tiles[c]
            if c + 1 < NCH:
                load_xc(c + 1)

            def proj_qk():
                # q heads + k, with RoPE applied out of PSUM
                for qh in range(QH + 1):
                    ps = ps_pj.tile([P, CH], f32, tag="proj")
                    for kt in range(KT):
                        lhs = (
                            wq_sb[:, kt, qh * P:(qh + 1) * P]
                            if qh < QH
                            else wk_sb[:, kt, :]
                        )
                        nc.tensor.matmul(
                            ps, lhs, xc[:, kt, :], start=(kt == 0), stop=(kt == KT - 1)
                        )
                    qf = work.tile([P, CH], f32, tag="qf")
                    nc.scalar.copy(out=qf, in_=ps)
                    rot = work.tile([P, CH], f32, tag="rot")
                    nc.scalar.dma_start(out=rot[0:64, :], in_=qf[64:128, :])
                    nc.scalar.dma_start(out=rot[64:128, :], in_=qf[0:64, :])
                    t1 = work.tile([P, CH], f32, tag="t1")
                    nc.vector.tensor_mul(t1, qf, cos_sb[:, cs])
                    t2 = work.tile([P, CH], f32, tag="t2")
                    nc.vector.tensor_mul(t2, rot, sin_sb[:, cs])
                    dst = qro[:, qh, cs] if qh < QH else kro[:, cs]
                    nc.vector.tensor_add(dst, t1, t2)

            def proj_v():
                # v in straight layout [s, d]
                for st in range(ST):
                    s0 = c * ST + st
                    ps = ps_pj.tile([P, P], f32, tag="proj")
                    for kt in range(KT):
                        nc.tensor.matmul(
                            ps,
                            xc[:, kt, st * P:(st + 1) * P],
                            wv_sb[:, kt, :],
                            start=(kt == 0),
                            stop=(kt == KT - 1),
                        )
                    nc.vector.tensor_copy(out=v_sb[:, s0, :], in_=ps)

            def proj_gate():
                # gate heads: sigmoid(x @ Wg + bg), transposed layout
                for qh in range(QH):
                    ps = ps_pj.tile([P, CH], f32, tag="proj")
                    for kt in range(KT):
                        nc.tensor.matmul(
                            ps,
                            wg_sb[:, kt, qh * P:(qh + 1) * P],
                            xc[:, kt, :],
                            start=(kt == 0),
                            stop=(kt == KT - 1),
                        )
                    th = work.tile([P, CH], bf16, tag="gth", bufs=2)
                    nc.scalar.activation(
                        out=th,
                        in_=ps,
                        func=tanh,
                        bias=bg_sb[:, qh:qh + 1],
                        scale=0.5,
                    )
                    nc.vector.tensor_scalar(
                        out=gt[:, qh, cs], in0=th,
                        scalar1=0.5, scalar2=0.5,
                        op0=mybir.AluOpType.mult, op1=mybir.AluOpType.add,
                    )

            if c == 0:
                # wg is near the end of the weight-load queue: q/k first
                proj_qk(); proj_v(); proj_gate()
            else:
                # gate first: the sigmoid table swap overlaps gate matmuls
                # instead of blocking the q-proj PSUM recycle
                proj_gate(); proj_qk(); proj_v()

            # ---- attention for this sq chunk, heads in pairs ----
            ag = agp.tile([P, QH, CH], bf16, tag="ag")
            ntiles = (c + 1) * ST

            def norm_bc(rc, name):
                bc = ps_sc.tile([P, CH], f32, tag="sc", name=name)
                nc.tensor.matmul(bc, ones_bc, rc, start=True, stop=True)
                return bc

            def norm_t3(qh, at):
                # PSUM x SBUF (gate) first — frees the at bank early and
                # keeps both muls to a single PSUM operand each.
                t3 = work.tile([P, CH], f32, tag="t3", bufs=3)
                nc.vector.tensor_mul(t3, at, gt[:, qh, cs])
                return t3

            def norm_ag(qh, t3, bc):
                nc.vector.tensor_mul(ag[:, qh, :], t3, bc)

            def attn_tile(t, qh, at, dn, sc_name):
                o = t - c * ST
                off = o * P if o > 0 else 0  # live cols of diagonal tiles
                sc_ps = ps_sc.tile([P, CH], f32, tag="sc", name=sc_name)
                nc.tensor.matmul(
                    sc_ps[:, off:],
                    kro[:, t * P:(t + 1) * P],
                    qro[:, qh, c * CH + off:(c + 1) * CH],
                    start=True,
                    stop=True,
                )
                pr = work.tile([P, CH], bf16, tag="probs", bufs=4)
                nc.scalar.activation(
                    out=pr[:, off:], in_=sc_ps[:, off:], func=expf, scale=SCALE
                )
                if o >= 0:
                    nc.vector.tensor_mul(pr[:, off:], pr[:, off:], msk_sb[:, o, off:])
                return pr, off

            def attn_accum(t, pr, off, at, dn, dnst):
                nc.tensor.matmul(
                    at[:, off:], v_sb[:, t, :], pr[:, off:],
                    start=(t == 0), stop=(t == ntiles - 1),
                )
                # denominator: merge pairs of full-width tiles on the DVE so
                # one ones-matmul covers two probability tiles
                diag = t - c * ST >= 0
                if diag:
                    if dnst["pr"] is not None:
                        nc.tensor.matmul(
                            dn, ones_pv, dnst["pr"],
                            start=dnst["first"], stop=False,
                        )
                        dnst["pr"] = None
                        dnst["first"] = False
                    nc.tensor.matmul(
                        dn[:, off:], ones_pv, pr[:, off:],
                        start=dnst["first"], stop=(t == ntiles - 1),
                    )
                    dnst["first"] = False
                elif dnst["pr"] is None:
                    dnst["pr"] = pr
                else:
                    m = work.tile([P, CH], bf16, tag="dmrg", bufs=2)
                    nc.vector.tensor_add(m, dnst["pr"], pr)
                    nc.tensor.matmul(
                        dn, ones_pv, m, start=dnst["first"], stop=False,
                    )
                    dnst["pr"] = None
                    dnst["first"] = False

            pend = []  # [(qh, at, rc)] awaiting normalization
            for ha, hb in ((0, 1), (2, 3)):
                at_a = ps_pj.tile([P, CH], f32, tag="proj", name="at_a")
                at_b = ps_pj.tile([P, CH], f32, tag="proj", name="at_b")
                dn_a = ps_dn.tile([1, CH], f32, tag="dn", name="dn_a")
                dn_b = ps_dn.tile([1, CH], f32, tag="dn", name="dn_b")
                dnst_a = {"pr": None, "first": True}
                dnst_b = {"pr": None, "first": True}
                for t in range(ntiles):
                    pr_a, off = attn_tile(t, ha, at_a, dn_a, "sc_a")
                    pr_b, _ = attn_tile(t, hb, at_b, dn_b, "sc_b")
                    # normalize the previous pair inside the exp-latency
                    # bubble of this pair's first two tiles
                    if t < 2 and pend:
                        qh_p, at_p, rc_p = pend.pop(0)
                        t3_p = norm_t3(qh_p, at_p)
                        norm_ag(qh_p, t3_p, norm_bc(rc_p, f"bc{qh_p}"))
                    attn_accum(t, pr_a, off, at_a, dn_a, dnst_a)
                    attn_accum(t, pr_b, off, at_b, dn_b, dnst_b)
                rc_a = work.tile([1, CH], f32, tag="recip", bufs=4)
                nc.vector.reciprocal_approx_fast(out=rc_a, in_=dn_a)
                rc_b = work.tile([1, CH], f32, tag="recip", bufs=4)
                nc.vector.reciprocal_approx_fast(out=rc_b, in_=dn_b)
                pend += [(ha, at_a, rc_a), (hb, at_b, rc_b)]

            # o_proj of the previous chunk, with the last pair's
            # normalization woven into the first few groups.
            self_norm = pend
            pend = []
            if ag_prev is not None:
                emit_oproj(nc, ps_pj, outp, out, wo_sb, ag_prev, self_norm,
                           norm_bc, norm_t3, norm_ag)
            else:
                t3s = [norm_t3(qh, at) for qh, at, _ in self_norm]
                bcs = [norm_bc(rc, f"bc{qh}") for qh, _, rc in self_norm]
                for (qh, _, _), t3, bc in zip(self_norm, t3s, bcs):
                    norm_ag(qh, t3, bc)
            ag_prev = (c, ag)

        emit_oproj(nc, ps_pj, outp, out, wo_sb, ag_prev, [], None, None, None)

    nc.finalize()
    return nc


def emit_oproj(nc, ps_pj, outp, out, wo_sb, ag_info, norm2,
               norm_bc, norm_t3, norm_ag):
    f32 = mybir.dt.float32
    f16 = mybir.dt.float16
    c, ag = ag_info
    ST = CH // P
    bcs = []
    t3s = []
    gi = 0
    for st in range(ST):
        r0 = c * CH + st * P
        for h0 in range(HIDDEN // CH):
            if gi == 0 and norm2:
                # t3 muls free the at banks that groups 2/3 will reuse;
                # they only need the gate, so they run during group 0/1
                t3s = [norm_t3(qh, at) for qh, at, _ in norm2]
            if gi == 2 and norm2:
                # bc matmuls for the last pair (reciprocals are long done)
                bcs = [norm_bc(rc, f"bc{qh}") for qh, _, rc in norm2]
            if gi == 4 and norm2:
                for (qh, _, _), t3, bc in zip(norm2, t3s, bcs):
                    norm_ag(qh, t3, bc)
            ps = ps_pj.tile([P, CH], f32, tag="proj", name=f"op{gi}")
            for dt in range(QH):
                nc.tensor.matmul(
                    ps,
                    ag[:, dt, st * P:(st + 1) * P],
                    wo_sb[:, dt, h0 * CH:(h0 + 1) * CH],
                    start=(dt == 0),
                    stop=(dt == QH - 1),
                )
            ob = outp.tile([P, CH], f16, tag="ob")
            nc.vector.tensor_copy(out=ob, in_=ps)
            nc.sync.dma_start(out=out[r0:r0 + P, h0 * CH:(h0 + 1) * CH], in_=ob)
            gi += 1


_PROGRAMS = {}


def _get_program(S=S_FULL):
    if S not in _PROGRAMS:
        _PROGRAMS[S] = build_program(S)
    return _PROGRAMS[S]


def _host_tables(position_ids_b, S):
    pos = np.asarray(position_ids_b, dtype=np.float32)  # [S]
    inv = 1.0 / (ROPE_THETA ** (np.arange(0, P, 2, dtype=np.float32) / P))  # [64]
    ang = np.concatenate([inv, inv]).astype(np.float32)[:, None] * pos[None, :]
    cosT = np.cos(ang).astype(BF16)
    sgn = np.where(np.arange(P) < 64, -1.0, 1.0).astype(np.float32)
    sinT = (np.sin(ang) * sgn[:, None]).astype(BF16)
    return cosT, sinT


def _causal_masks():
    o = np.arange(CH // P)[:, None, None]
    r = np.arange(P)[None, :, None]
    j = np.arange(CH)[None, None, :]
    return ((P * o + r) <= j).astype(BF16)


def make_in_maps(x, position_ids, Wq, Wk, Wv, Wo, Wg, bg, S=S_FULL):
    x = np.asarray(x, dtype=np.float32)
    msk = _causal_masks()
    maps = []
    xT_b = [np.ascontiguousarray(x[b, :S].T).astype(BF16) for b in range(B)]
    tabs = [_host_tables(np.asarray(position_ids)[b, :S], S) for b in range(B)]
    Wq = np.asarray(Wq, np.float32)
    Wk = np.asarray(Wk, np.float32)
    Wv = np.asarray(Wv, np.float32)
    Wo = np.asarray(Wo, np.float32)
    Wg = np.asarray(Wg, np.float32)
    bg = np.asarray(bg, np.float32)
    for core in range(8):
        b, g = core // 4, core % 4
        cosT, sinT = tabs[b]
        maps.append({
            "xT": xT_b[b],
            "wq": np.ascontiguousarray(Wq[:, g * DQ:(g + 1) * DQ]).astype(BF16),
            "wk": np.ascontiguousarray(Wk[:, g * P:(g + 1) * P]).astype(BF16),
            "wv": np.ascontiguousarray(Wv[:, g * P:(g + 1) * P]).astype(BF16),
            "wg": np.ascontiguousarray(Wg[:, g * DQ:(g + 1) * DQ]).astype(BF16),
            "wo": np.ascontiguousarray(Wo[g * DQ:(g + 1) * DQ, :]).astype(BF16),
            "bg": np.ascontiguousarray(bg[g * DQ:(g + 1) * DQ]) * np.float32(0.5),
            "cosT": cosT,
            "sinT": sinT,
            "msk": msk,
        })
    return maps


def run(inputs, S=S_FULL, trace=False, **kw):
    nc = _get_program(S)
    maps = make_in_maps(S=S, **inputs)
    res = run_bass_kernel_spmd(nc, maps, core_ids=list(range(8)), trace=trace, **kw)
    out = np.zeros((B, S, HIDDEN), np.float32)
    for core in range(8):
        out[core // 4] += np.asarray(res.results[core]["out"], np.float32)
    return out, res


def kernel(x, position_ids, Wq, Wk, Wv, Wo, Wg, bg):
    out, _ = run(dict(x=x, position_ids=position_ids, Wq=Wq, Wk=Wk, Wv=Wv,
                      Wo=Wo, Wg=Wg, bg=bg))
    return out


# revision 16
# speedup vs baseline: 1.0562x; 1.0035x over previous
"""Trainium2 Bass kernel for LuluAttention (gated GQA attention + RoPE).

Sharding over 8 NeuronCores: core = b*4 + g where b = batch (2), g = head
group (4). Each core computes 4 Q heads + their shared KV head for one batch
element, plus the matching gate slice, and a partial o_proj output
(contraction over its 512 attn dims). Host sums the 4 partials per batch.

All on-chip tensors are kept in transposed layout ([dim, seq]) so the
attention pipeline needs no on-chip transposes:
  qT/kT [d, s]  -> scoresT[sk, sq] = kT_tile.T @ qT_chunk
  softmax over sk (partition dim): denominator via ones-matmul, broadcast of
  the reciprocal via a K=1 matmul.
  v kept straight [s, d] -> attnT[d, sq] = v_tile.T @ probsT
  agT[d, sq] = attnT * recip * gateT  feeds o_proj directly as lhsT.
RoPE rotate-half needs a cross-partition rotation by 64: done with two DMA
copies, signs folded into the host-precomputed sin table.

Perf structure (v3):
  - Dummy matmuls at kernel start keep the PE busy while the first weight/x
    DMAs land, so HAM is warm when real work starts.
  - Weight DMAs issue on the sync queue in first-use order; the RoPE rotate
    DMAs go through the scalar (ACT) DGE queue so they never sit behind
    megabytes of weight traffic.
  - Attention processes heads in pairs: the two heads' score/av/denominator
    tiles interleave, hiding the exp (scalar engine) latency.
  - Softmax normalization (reciprocal -> broadcast matmul -> muls) for each
    head pair is deferred into the next pair / the o_proj stream so the PE
    never waits on the DVE reciprocal.
  - o_proj of chunk c is issued after attention of chunk c+1; its PSUM
    groups cycle a 4-deep ring shared with the projection and attention
    accumulators, and its output casts run on the otherwise-idle GpSimd.
  - Diagonal causal tiles only compute the live column range.
  - fp16 partial outputs (host accumulates in fp32), bf16 rope tables.
"""

import numpy as np
import ml_dtypes
from contextlib import ExitStack

import concourse.bass as bass
import concourse.bacc as bacc
import concourse.tile as tile
from concourse import mybir
from concourse.bass_utils import run_bass_kernel_spmd

BF16 = ml_dtypes.bfloat16

HIDDEN = 2048
B = 2
S_FULL = 2048
P = 128
CH = 512               # seq chunk width
QH = 4                 # q heads per core
DQ = QH * P            # 512 q dims per core
KT = HIDDEN // P       # 16 contraction tiles
SCALE = 1.0 / float(np.sqrt(128.0))
ROPE_THETA = 10000.0
WARMUP_MM = 40


def build_program(S=S_FULL):
    f32 = mybir.dt.float32
    f16 = mybir.dt.float16
    bf16 = mybir.dt.bfloat16
    tanh = mybir.ActivationFunctionType.Tanh
    expf = mybir.ActivationFunctionType.Exp

    NCH = S // CH
    ST = CH // P           # 4 seq sub-tiles per chunk

    nc = bacc.Bacc("TRN2", debug=False, target_bir_lowering=False)

    xT = nc.declare_dram_parameter("xT", [HIDDEN, S], bf16, False)
    wq = nc.declare_dram_parameter("wq", [HIDDEN, DQ], bf16, False)
    wk = nc.declare_dram_parameter("wk", [HIDDEN, P], bf16, False)
    wv = nc.declare_dram_parameter("wv", [HIDDEN, P], bf16, False)
    wg = nc.declare_dram_parameter("wg", [HIDDEN, DQ], bf16, False)
    wo = nc.declare_dram_parameter("wo", [DQ, HIDDEN], bf16, False)
    bg = nc.declare_dram_parameter("bg", [DQ], f32, False)
    cosT = nc.declare_dram_parameter("cosT", [P, S], bf16, False)
    sinT = nc.declare_dram_parameter("sinT", [P, S], bf16, False)
    msk = nc.declare_dram_parameter("msk", [ST, P, CH], bf16, False)
    out = nc.declare_dram_parameter("out", [S, HIDDEN], f16, True)

    with tile.TileContext(nc) as tc, ExitStack() as ctx:
        wpool = ctx.enter_context(tc.tile_pool(name="weights", bufs=1))
        xpool = ctx.enter_context(tc.tile_pool(name="xchunks", bufs=2))
        qkv = ctx.enter_context(tc.tile_pool(name="qkv", bufs=1))
        work = ctx.enter_context(tc.tile_pool(name="work", bufs=4))
        agp = ctx.enter_context(tc.tile_pool(name="agp", bufs=2))
        outp = ctx.enter_context(tc.tile_pool(name="outp", bufs=4))
        # PSUM: 4 + 2 + 2 = 8 banks.
        ps_pj = ctx.enter_context(tc.tile_pool(name="ps_pj", bufs=4, space="PSUM"))
        ps_sc = ctx.enter_context(tc.tile_pool(name="ps_sc", bufs=2, space="PSUM"))
        ps_dn = ctx.enter_context(tc.tile_pool(name="ps_dn", bufs=2, space="PSUM"))

        # ---- persistent loads, ordered by first use (sync DGE queue) ----
        wq_sb = wpool.tile([P, KT, DQ], bf16, tag="wq")
        nc.sync.dma_start(
            out=wq_sb[:, :, 0:P],
            in_=wq[:, 0:P].rearrange("(kt p) n -> p kt n", p=P),
        )

        xc_tiles = [None] * NCH

        def load_xc(c):
            t = xpool.tile([P, KT, CH], bf16, tag="xc", name=f"xc{c}")
            nc.sync.dma_start(
                out=t, in_=xT[:, c * CH:(c + 1) * CH].rearrange("(kt p) n -> p kt n", p=P)
            )
            xc_tiles[c] = t

        load_xc(0)

        nc.sync.dma_start(
            out=wq_sb[:, :, P:DQ],
            in_=wq[:, P:DQ].rearrange("(kt p) n -> p kt n", p=P),
        )
        wk_sb = wpool.tile([P, KT, P], bf16, tag="wk")
        nc.sync.dma_start(out=wk_sb, in_=wk[:, :].rearrange("(kt p) n -> p kt n", p=P))
        wv_sb = wpool.tile([P, KT, P], bf16, tag="wv")
        nc.sync.dma_start(out=wv_sb, in_=wv[:, :].rearrange("(kt p) n -> p kt n", p=P))
        cos_sb = wpool.tile([P, S], bf16, tag="cos")
        nc.sync.dma_start(out=cos_sb, in_=cosT[:, :])
        sin_sb = wpool.tile([P, S], bf16, tag="sin")
        nc.sync.dma_start(out=sin_sb, in_=sinT[:, :])
        msk_sb = wpool.tile([P, ST, CH], bf16, tag="msk")
        nc.sync.dma_start(out=msk_sb, in_=msk[:, :, :].rearrange("o p n -> p o n"))
        bg_sb = wpool.tile([P, QH], f32, tag="bg")
        nc.sync.dma_start(out=bg_sb, in_=bg[:].rearrange("(h p) -> p h", p=P))
        wg_sb = wpool.tile([P, KT, DQ], bf16, tag="wg")
        nc.sync.dma_start(out=wg_sb, in_=wg[:, :].rearrange("(kt p) n -> p kt n", p=P))
        wo_sb = wpool.tile([P, QH, HIDDEN], bf16, tag="wo")
        nc.sync.dma_start(out=wo_sb, in_=wo[:, :].rearrange("(dt p) n -> p dt n", p=P))

        ones_pv = wpool.tile([P, 1], bf16, tag="ones_pv")
        nc.vector.memset(ones_pv, 1.0)
        ones_bc = wpool.tile([1, P], f32, tag="ones_bc")
        nc.vector.memset(ones_bc, 1.0)

        # ---- HAM warmup: keep PE busy while the first DMAs land ----
        warm_in = wpool.tile([P, CH], bf16, tag="warm")
        nc.vector.memset(warm_in, 0.0)
        for i in range(WARMUP_MM):
            wps = ps_sc.tile([P, CH], f32, tag="sc", name=f"warm{i}")
            nc.tensor.matmul(wps, warm_in[:, 0:P], warm_in, start=True, stop=True)

        # persistent per-core activations (transposed layouts)
        qro = qkv.tile([P, QH, S], bf16, tag="qro")
        kro = qkv.tile([P, S], bf16, tag="kro")
        v_sb = qkv.tile([P, S // P, P], bf16, tag="v")
        gt = qkv.tile([P, QH, S], bf16, tag="gt")

        ag_prev = None  # (chunk_idx, ag_tile)

        for c in range(NCH):
            cs = slice(c * CH, (c + 1) * CH)
            xc = xc_tiles[c]
            if c + 1 < NCH:
                load_xc(c + 1)

            def proj_qk():
                # q heads + k, with RoPE applied out of PSUM
                for qh in range(QH + 1):
                    ps = ps_pj.tile([P, CH], f32, tag="proj")
                    for kt in range(KT):
                        lhs = (
                            wq_sb[:, kt, qh * P:(qh + 1) * P]
                            if qh < QH
                            else wk_sb[:, kt, :]
                        )
                        nc.tensor.matmul(
                            ps, lhs, xc[:, kt, :], start=(kt == 0), stop=(kt == KT - 1)
                        )
                    qf = work.tile([P, CH], f32, tag="qf")
                    nc.scalar.copy(out=qf, in_=ps)
                    rot = work.tile([P, CH], f32, tag="rot")
                    nc.scalar.dma_start(out=rot[0:64, :], in_=qf[64:128, :])
                    nc.scalar.dma_start(out=rot[64:128, :], in_=qf[0:64, :])
                    t1 = work.tile([P, CH], f32, tag="t1")
                    nc.vector.tensor_mul(t1, qf, cos_sb[:, cs])
                    t2 = work.tile([P, CH], f32, tag="t2")
                    nc.vector.tensor_mul(t2, rot, sin_sb[:, cs])
                    dst = qro[:, qh, cs] if qh < QH else kro[:, cs]
                    nc.vector.tensor_add(dst, t1, t2)

            def proj_v():
                # v in straight layout [s, d]
                for st in range(ST):
                    s0 = c * ST + st
                    ps = ps_pj.tile([P, P], f32, tag="proj")
                    for kt in range(KT):
                        nc.tensor.matmul(
                            ps,
                            xc[:, kt, st * P:(st + 1) * P],
                            wv_sb[:, kt, :],
                            start=(kt == 0),
                            stop=(kt == KT - 1),
                        )
                    if c == 0:
                        nc.scalar.copy(out=v_sb[:, s0, :], in_=ps)
                    else:
                        nc.vector.tensor_copy(out=v_sb[:, s0, :], in_=ps)

            def proj_gate():
                # gate heads: sigmoid(x @ Wg + bg), transposed layout
                for qh in range(QH):
                    ps = ps_pj.tile([P, CH], f32, tag="proj")
                    for kt in range(KT):
                        nc.tensor.matmul(
                            ps,
                            wg_sb[:, kt, qh * P:(qh + 1) * P],
                            xc[:, kt, :],
                            start=(kt == 0),
                            stop=(kt == KT - 1),
                        )
                    th = work.tile([P, CH], bf16, tag="gth", bufs=2)
                    nc.scalar.activation(
                        out=th,
                        in_=ps,
                        func=tanh,
                        bias=bg_sb[:, qh:qh + 1],
                        scale=0.5,
                    )
                    nc.vector.tensor_scalar(
                        out=gt[:, qh, cs], in0=th,
                        scalar1=0.5, scalar2=0.5,
                        op0=mybir.AluOpType.mult, op1=mybir.AluOpType.add,
                    )

            if c == 0:
                # wg is near the end of the weight-load queue: q/k first
                proj_qk(); proj_v(); proj_gate()
            else:
                # gate first: the sigmoid table swap overlaps gate matmuls
                # instead of blocking the q-proj PSUM recycle
                proj_gate(); proj_qk(); proj_v()

            # ---- attention for this sq chunk, heads in pairs ----
            ag = agp.tile([P, QH, CH], bf16, tag="ag")
            ntiles = (c + 1) * ST

            def norm_bc(rc, name):
                bc = ps_sc.tile([P, CH], f32, tag="sc", name=name)
                nc.tensor.matmul(bc, ones_bc, rc, start=True, stop=True)
                return bc

            def norm_t3(qh, at):
                # PSUM x SBUF (gate) first — frees the at bank early and
                # keeps both muls to a single PSUM operand each.
                t3 = work.tile([P, CH], f32, tag="t3", bufs=3)
                nc.vector.tensor_mul(t3, at, gt[:, qh, cs])
                return t3

            def norm_ag(qh, t3, bc):
                nc.vector.tensor_mul(ag[:, qh, :], t3, bc)

            def attn_tile(t, qh, at, dn, sc_name):
                o = t - c * ST
                off = o * P if o > 0 else 0  # live cols of diagonal tiles
                sc_ps = ps_sc.tile([P, CH], f32, tag="sc", name=sc_name)
                nc.tensor.matmul(
                    sc_ps[:, off:],
                    kro[:, t * P:(t + 1) * P],
                    qro[:, qh, c * CH + off:(c + 1) * CH],
                    start=True,
                    stop=True,
                )
                pr = work.tile([P, CH], bf16, tag="probs", bufs=4)
                nc.scalar.activation(
                    out=pr[:, off:], in_=sc_ps[:, off:], func=expf, scale=SCALE
                )
                if o >= 0:
                    nc.vector.tensor_mul(pr[:, off:], pr[:, off:], msk_sb[:, o, off:])
                return pr, off

            def attn_accum(t, pr, off, at, dn, dnst):
                nc.tensor.matmul(
                    at[:, off:], v_sb[:, t, :], pr[:, off:],
                    start=(t == 0), stop=(t == ntiles - 1),
                )
                # denominator: merge pairs of full-width tiles on the DVE so
                # one ones-matmul covers two probability tiles
                diag = t - c * ST >= 0
                if diag:
                    if dnst["pr"] is not None:
                        nc.tensor.matmul(
                            dn, ones_pv, dnst["pr"],
                            start=dnst["first"], stop=False,
                        )
                        dnst["pr"] = None
                        dnst["first"] = False
                    nc.tensor.matmul(
                        dn[:, off:], ones_pv, pr[:, off:],
                        start=dnst["first"], stop=(t == ntiles - 1),
                    )
                    dnst["first"] = False
                elif dnst["pr"] is None:
                    dnst["pr"] = pr
                else:
                    m = work.tile([P, CH], bf16, tag="dmrg", bufs=2)
                    nc.vector.tensor_add(m, dnst["pr"], pr)
                    nc.tensor.matmul(
                        dn, ones_pv, m, start=dnst["first"], stop=False,
                    )
                    dnst["pr"] = None
                    dnst["first"] = False

            pend = []  # [(qh, at, rc)] awaiting normalization
            for ha, hb in ((0, 1), (2, 3)):
                at_a = ps_pj.tile([P, CH], f32, tag="proj", name="at_a")
                at_b = ps_pj.tile([P, CH], f32, tag="proj", name="at_b")
                dn_a = ps_dn.tile([1, CH], f32, tag="dn", name="dn_a")
                dn_b = ps_dn.tile([1, CH], f32, tag="dn", name="dn_b")
                dnst_a = {"pr": None, "first": True}
                dnst_b = {"pr": None, "first": True}
                for t in range(ntiles):
                    pr_a, off = attn_tile(t, ha, at_a, dn_a, "sc_a")
                    pr_b, _ = attn_tile(t, hb, at_b, dn_b, "sc_b")
                    # normalize the previous pair inside the exp-latency
                    # bubble of this pair's first two tiles
                    if t < 2 and pend:
                        qh_p, at_p, rc_p = pend.pop(0)
                        t3_p = norm_t3(qh_p, at_p)
                        norm_ag(qh_p, t3_p, norm_bc(rc_p, f"bc{qh_p}"))
                    attn_accum(t, pr_a, off, at_a, dn_a, dnst_a)
                    attn_accum(t, pr_b, off, at_b, dn_b, dnst_b)
                rc_a = work.tile([1, CH], f32, tag="recip", bufs=4)
                nc.vector.reciprocal_approx_fast(out=rc_a, in_=dn_a)
                rc_b = work.tile([1, CH], f32, tag="recip", bufs=4)
                nc.vector.reciprocal_approx_fast(out=rc_b, in_=dn_b)
                pend += [(ha, at_a, rc_a), (hb, at_b, rc_b)]

            # o_proj of the previous chunk, with the last pair's
            # normalization woven into the first few groups.
            self_norm = pend
            pend = []
            if ag_prev is not None:
                emit_oproj(nc, ps_pj, outp, out, wo_sb, ag_prev, self_norm,
                           norm_bc, norm_t3, norm_ag)
            else:
                t3s = [norm_t3(qh, at) for qh, at, _ in self_norm]
                bcs = [norm_bc(rc, f"bc{qh}") for qh, _, rc in self_norm]
                for (qh, _, _), t3, bc in zip(self_norm, t3s, bcs):
                    norm_ag(qh, t3, bc)
            ag_prev = (c, ag)

        emit_oproj(nc, ps_pj, outp, out, wo_sb, ag_prev, [], None, None, None)

    nc.finalize()
    return nc


def emit_oproj(nc, ps_pj, outp, out, wo_sb, ag_info, norm2,
               norm_bc, norm_t3, norm_ag):
    f32 = mybir.dt.float32
    f16 = mybir.dt.float16
    c, ag = ag_info
    ST = CH // P
    bcs = []
    t3s = []
    gi = 0
    for st in range(ST):
        r0 = c * CH + st * P
        for h0 in range(HIDDEN // CH):
            if gi == 0 and norm2:
                # t3 muls free the at banks that groups 2/3 will reuse;
                # they only need the gate, so they run during group 0/1
                t3s = [norm_t3(qh, at) for qh, at, _ in norm2]
            if gi == 2 and norm2:
                # bc matmuls for the last pair (reciprocals are long done)
                bcs = [norm_bc(rc, f"bc{qh}") for qh, _, rc in norm2]
            if gi == 4 and norm2:
                for (qh, _, _), t3, bc in zip(norm2, t3s, bcs):
                    norm_ag(qh, t3, bc)
            ps = ps_pj.tile([P, CH], f32, tag="proj", name=f"op{gi}")
            for dt in range(QH):
                nc.tensor.matmul(
                    ps,
                    ag[:, dt, st * P:(st + 1) * P],
                    wo_sb[:, dt, h0 * CH:(h0 + 1) * CH],
                    start=(dt == 0),
                    stop=(dt == QH - 1),
                )
            ob = outp.tile([P, CH], f16, tag="ob")
            nc.vector.tensor_copy(out=ob, in_=ps)
            nc.sync.dma_start(out=out[r0:r0 + P, h0 * CH:(h0 + 1) * CH], in_=ob)
            gi += 1


_PROGRAMS = {}


def _get_program(S=S_FULL):
    if S not in _PROGRAMS:
        _PROGRAMS[S] = build_program(S)
    return _PROGRAMS[S]


def _host_tables(position_ids_b, S):
    pos = np.asarray(position_ids_b, dtype=np.float32)  # [S]
    inv = 1.0 / (ROPE_THETA ** (np.arange(0, P, 2, dtype=np.float32) / P))  # [64]
    ang = np.concatenate([inv, inv]).astype(np.float32)[:, None] * pos[None, :]
    cosT = np.cos(ang).astype(BF16)
    sgn = np.where(np.arange(P) < 64, -1.0, 1.0).astype(np.float32)
    sinT = (np.sin(ang) * sgn[:, None]).astype(BF16)
    return cosT, sinT


def _causal_masks():
    o = np.arange(CH // P)[:, None, None]
    r = np.arange(P)[None, :, None]
    j = np.arange(CH)[None, None, :]
    return ((P * o + r) <= j).astype(BF16)


def make_in_maps(x, position_ids, Wq, Wk, Wv, Wo, Wg, bg, S=S_FULL):
    x = np.asarray(x, dtype=np.float32)
    msk = _causal_masks()
    maps = []
    xT_b = [np.ascontiguousarray(x[b, :S].T).astype(BF16) for b in range(B)]
    tabs = [_host_tables(np.asarray(position_ids)[b, :S], S) for b in range(B)]
    Wq = np.asarray(Wq, np.float32)
    Wk = np.asarray(Wk, np.float32)
    Wv = np.asarray(Wv, np.float32)
    Wo = np.asarray(Wo, np.float32)
    Wg = np.asarray(Wg, np.float32)
    bg = np.asarray(bg, np.float32)
    for core in range(8):
        b, g = core // 4, core % 4
        cosT, sinT = tabs[b]
        maps.append({
            "xT": xT_b[b],
            "wq": np.ascontiguousarray(Wq[:, g * DQ:(g + 1) * DQ]).astype(BF16),
            "wk": np.ascontiguousarray(Wk[:, g * P:(g + 1) * P]).astype(BF16),
            "wv": np.ascontiguousarray(Wv[:, g * P:(g + 1) * P]).astype(BF16),
            "wg": np.ascontiguousarray(Wg[:, g * DQ:(g + 1) * DQ]).astype(BF16),
            "wo": np.ascontiguousarray(Wo[g * DQ:(g + 1) * DQ, :]).astype(BF16),
            "bg": np.ascontiguousarray(bg[g * DQ:(g + 1) * DQ]) * np.float32(0.5),
            "cosT": cosT,
            "sinT": sinT,
            "msk": msk,
        })
    return maps


def run(inputs, S=S_FULL, trace=False, **kw):
    nc = _get_program(S)
    maps = make_in_maps(S=S, **inputs)
    res = run_bass_kernel_spmd(nc, maps, core_ids=list(range(8)), trace=trace, **kw)
    out = np.zeros((B, S, HIDDEN), np.float32)
    for core in range(8):
        out[core // 4] += np.asarray(res.results[core]["out"], np.float32)
    return out, res


def kernel(x, position_ids, Wq, Wk, Wv, Wo, Wg, bg):
    out, _ = run(dict(x=x, position_ids=position_ids, Wq=Wq, Wk=Wk, Wv=Wv,
                      Wo=Wo, Wg=Wg, bg=bg))
    return out


# revision 17
# speedup vs baseline: 1.0879x; 1.0300x over previous
"""Trainium2 Bass kernel for LuluAttention (gated GQA attention + RoPE).

Sharding over 8 NeuronCores: core = b*4 + g where b = batch (2), g = head
group (4). Each core computes 4 Q heads + their shared KV head for one batch
element, plus the matching gate slice, and a partial o_proj output
(contraction over its 512 attn dims). Host sums the 4 partials per batch.

All on-chip tensors are kept in transposed layout ([dim, seq]) so the
attention pipeline needs no on-chip transposes:
  qT/kT [d, s]  -> scoresT[sk, sq] = kT_tile.T @ qT_chunk
  softmax over sk (partition dim): denominator via ones-matmul, broadcast of
  the reciprocal via a K=1 matmul.
  v kept straight [s, d] -> attnT[d, sq] = v_tile.T @ probsT
  agT[d, sq] = attnT * recip * gateT  feeds o_proj directly as lhsT.
RoPE rotate-half needs a cross-partition rotation by 64: done with two DMA
copies, signs folded into the host-precomputed sin table.

Perf structure (v3):
  - Dummy matmuls at kernel start keep the PE busy while the first weight/x
    DMAs land, so HAM is warm when real work starts.
  - Weight DMAs issue on the sync queue in first-use order; the RoPE rotate
    DMAs go through the scalar (ACT) DGE queue so they never sit behind
    megabytes of weight traffic.
  - Attention processes heads in pairs: the two heads' score/av/denominator
    tiles interleave, hiding the exp (scalar engine) latency.
  - Softmax normalization (reciprocal -> broadcast matmul -> muls) for each
    head pair is deferred into the next pair / the o_proj stream so the PE
    never waits on the DVE reciprocal.
  - o_proj of chunk c is issued after attention of chunk c+1; its PSUM
    groups cycle a 4-deep ring shared with the projection and attention
    accumulators, and its output casts run on the otherwise-idle GpSimd.
  - Diagonal causal tiles only compute the live column range.
  - fp16 partial outputs (host accumulates in fp32), bf16 rope tables.
"""

import numpy as np
import ml_dtypes
from contextlib import ExitStack

import concourse.bass as bass
import concourse.bacc as bacc
import concourse.tile as tile
from concourse import mybir
from concourse.bass_utils import run_bass_kernel_spmd

BF16 = ml_dtypes.bfloat16

HIDDEN = 2048
B = 2
S_FULL = 2048
P = 128
CH = 512               # seq chunk width
QH = 4                 # q heads per core
DQ = QH * P            # 512 q dims per core
KT = HIDDEN // P       # 16 contraction tiles
SCALE = 1.0 / float(np.sqrt(128.0))
ROPE_THETA = 10000.0
WARMUP_MM = 46


def build_program(S=S_FULL):
    f32 = mybir.dt.float32
    f16 = mybir.dt.float16
    bf16 = mybir.dt.bfloat16
    tanh = mybir.ActivationFunctionType.Tanh
    expf = mybir.ActivationFunctionType.Exp

    NCH = S // CH
    ST = CH // P           # 4 seq sub-tiles per chunk

    nc = bacc.Bacc("TRN2", debug=False, target_bir_lowering=False)

    xT = nc.declare_dram_parameter("xT", [HIDDEN, S], bf16, False)
    wq = nc.declare_dram_parameter("wq", [HIDDEN, DQ], bf16, False)
    wk = nc.declare_dram_parameter("wk", [HIDDEN, P], bf16, False)
    wv = nc.declare_dram_parameter("wv", [HIDDEN, P], bf16, False)
    wg = nc.declare_dram_parameter("wg", [HIDDEN, DQ], bf16, False)
    wo = nc.declare_dram_parameter("wo", [DQ, HIDDEN], bf16, False)
    bg = nc.declare_dram_parameter("bg", [DQ], f32, False)
    cosT = nc.declare_dram_parameter("cosT", [P, S], bf16, False)
    sinT = nc.declare_dram_parameter("sinT", [P, S], bf16, False)
    msk = nc.declare_dram_parameter("msk", [ST, P, CH], bf16, False)
    out = nc.declare_dram_parameter("out", [S, HIDDEN], f16, True)

    with tile.TileContext(nc) as tc, ExitStack() as ctx:
        wpool = ctx.enter_context(tc.tile_pool(name="weights", bufs=1))
        xpool = ctx.enter_context(tc.tile_pool(name="xchunks", bufs=2))
        qkv = ctx.enter_context(tc.tile_pool(name="qkv", bufs=1))
        work = ctx.enter_context(tc.tile_pool(name="work", bufs=4))
        agp = ctx.enter_context(tc.tile_pool(name="agp", bufs=2))
        outp = ctx.enter_context(tc.tile_pool(name="outp", bufs=4))
        # PSUM: 4 + 2 + 2 = 8 banks.
        ps_pj = ctx.enter_context(tc.tile_pool(name="ps_pj", bufs=4, space="PSUM"))
        ps_sc = ctx.enter_context(tc.tile_pool(name="ps_sc", bufs=2, space="PSUM"))
        ps_dn = ctx.enter_context(tc.tile_pool(name="ps_dn", bufs=2, space="PSUM"))

        # ---- persistent loads, ordered by first use (sync DGE queue) ----
        wq_sb = wpool.tile([P, KT, DQ], bf16, tag="wq")
        nc.sync.dma_start(
            out=wq_sb[:, :, 0:P],
            in_=wq[:, 0:P].rearrange("(kt p) n -> p kt n", p=P),
        )

        xc_tiles = [None] * NCH

        def load_xc(c):
            t = xpool.tile([P, KT, CH], bf16, tag="xc", name=f"xc{c}")
            nc.sync.dma_start(
                out=t, in_=xT[:, c * CH:(c + 1) * CH].rearrange("(kt p) n -> p kt n", p=P)
            )
            xc_tiles[c] = t

        load_xc(0)

        nc.sync.dma_start(
            out=wq_sb[:, :, P:DQ],
            in_=wq[:, P:DQ].rearrange("(kt p) n -> p kt n", p=P),
        )
        wk_sb = wpool.tile([P, KT, P], bf16, tag="wk")
        nc.sync.dma_start(out=wk_sb, in_=wk[:, :].rearrange("(kt p) n -> p kt n", p=P))
        wv_sb = wpool.tile([P, KT, P], bf16, tag="wv")
        nc.sync.dma_start(out=wv_sb, in_=wv[:, :].rearrange("(kt p) n -> p kt n", p=P))
        wg_sb = wpool.tile([P, KT, DQ], bf16, tag="wg")
        nc.sync.dma_start(out=wg_sb, in_=wg[:, :].rearrange("(kt p) n -> p kt n", p=P))
        bg_sb = wpool.tile([P, QH], f32, tag="bg")
        nc.sync.dma_start(out=bg_sb, in_=bg[:].rearrange("(h p) -> p h", p=P))
        cos_sb = wpool.tile([P, S], bf16, tag="cos")
        nc.sync.dma_start(out=cos_sb, in_=cosT[:, :])
        sin_sb = wpool.tile([P, S], bf16, tag="sin")
        nc.sync.dma_start(out=sin_sb, in_=sinT[:, :])
        msk_sb = wpool.tile([P, ST, CH], bf16, tag="msk")
        nc.sync.dma_start(out=msk_sb, in_=msk[:, :, :].rearrange("o p n -> p o n"))
        wo_sb = wpool.tile([P, QH, HIDDEN], bf16, tag="wo")
        nc.sync.dma_start(out=wo_sb, in_=wo[:, :].rearrange("(dt p) n -> p dt n", p=P))

        ones_pv = wpool.tile([P, 1], bf16, tag="ones_pv")
        nc.vector.memset(ones_pv, 1.0)
        ones_bc = wpool.tile([1, P], f32, tag="ones_bc")
        nc.vector.memset(ones_bc, 1.0)

        # ---- HAM warmup: keep PE busy while the first DMAs land ----
        warm_in = wpool.tile([P, CH], bf16, tag="warm")
        nc.vector.memset(warm_in, 0.0)
        for i in range(WARMUP_MM):
            wps = ps_sc.tile([P, CH], f32, tag="sc", name=f"warm{i}")
            nc.tensor.matmul(wps, warm_in[:, 0:P], warm_in, start=True, stop=True)

        # persistent per-core activations (transposed layouts)
        qro = qkv.tile([P, QH, S], bf16, tag="qro")
        kro = qkv.tile([P, S], bf16, tag="kro")
        v_sb = qkv.tile([P, S // P, P], bf16, tag="v")
        gt = qkv.tile([P, QH, S], bf16, tag="gt")

        ag_prev = None  # (chunk_idx, ag_tile)

        for c in range(NCH):
            cs = slice(c * CH, (c + 1) * CH)
            xc = xc_tiles[c]
            if c + 1 < NCH:
                load_xc(c + 1)

            def proj_qk():
                # q heads + k, with RoPE applied out of PSUM
                for qh in range(QH + 1):
                    ps = ps_pj.tile([P, CH], f32, tag="proj")
                    for kt in range(KT):
                        lhs = (
                            wq_sb[:, kt, qh * P:(qh + 1) * P]
                            if qh < QH
                            else wk_sb[:, kt, :]
                        )
                        nc.tensor.matmul(
                            ps, lhs, xc[:, kt, :], start=(kt == 0), stop=(kt == KT - 1)
                        )
                    qf = work.tile([P, CH], f32, tag="qf")
                    nc.scalar.copy(out=qf, in_=ps)
                    rot = work.tile([P, CH], f32, tag="rot")
                    nc.scalar.dma_start(out=rot[0:64, :], in_=qf[64:128, :])
                    nc.scalar.dma_start(out=rot[64:128, :], in_=qf[0:64, :])
                    t1 = work.tile([P, CH], f32, tag="t1")
                    nc.vector.tensor_mul(t1, qf, cos_sb[:, cs])
                    t2 = work.tile([P, CH], f32, tag="t2")
                    nc.vector.tensor_mul(t2, rot, sin_sb[:, cs])
                    dst = qro[:, qh, cs] if qh < QH else kro[:, cs]
                    nc.vector.tensor_add(dst, t1, t2)

            def proj_v():
                # v in straight layout [s, d]
                for st in range(ST):
                    s0 = c * ST + st
                    ps = ps_pj.tile([P, P], f32, tag="proj")
                    for kt in range(KT):
                        nc.tensor.matmul(
                            ps,
                            xc[:, kt, st * P:(st + 1) * P],
                            wv_sb[:, kt, :],
                            start=(kt == 0),
                            stop=(kt == KT - 1),
                        )
                    if c == 0:
                        nc.scalar.copy(out=v_sb[:, s0, :], in_=ps)
                    else:
                        nc.vector.tensor_copy(out=v_sb[:, s0, :], in_=ps)

            def proj_gate():
                # gate heads: sigmoid(x @ Wg + bg), transposed layout
                for qh in range(QH):
                    ps = ps_pj.tile([P, CH], f32, tag="proj")
                    for kt in range(KT):
                        nc.tensor.matmul(
                            ps,
                            wg_sb[:, kt, qh * P:(qh + 1) * P],
                            xc[:, kt, :],
                            start=(kt == 0),
                            stop=(kt == KT - 1),
                        )
                    th = work.tile([P, CH], bf16, tag="gth", bufs=2)
                    nc.scalar.activation(
                        out=th,
                        in_=ps,
                        func=tanh,
                        bias=bg_sb[:, qh:qh + 1],
                        scale=0.5,
                    )
                    nc.vector.tensor_scalar(
                        out=gt[:, qh, cs], in0=th,
                        scalar1=0.5, scalar2=0.5,
                        op0=mybir.AluOpType.mult, op1=mybir.AluOpType.add,
                    )

            if c == 0:
                # wg is near the end of the weight-load queue: q/k first
                proj_qk(); proj_v(); proj_gate()
            else:
                # gate first: the sigmoid table swap overlaps gate matmuls
                # instead of blocking the q-proj PSUM recycle
                proj_gate(); proj_qk(); proj_v()

            # ---- attention for this sq chunk, heads in pairs ----
            ag = agp.tile([P, QH, CH], bf16, tag="ag")
            ntiles = (c + 1) * ST

            def norm_bc(rc, name):
                bc = ps_sc.tile([P, CH], f32, tag="sc", name=name)
                nc.tensor.matmul(bc, ones_bc, rc, start=True, stop=True)
                return bc

            def norm_t3(qh, at):
                # PSUM x SBUF (gate) first — frees the at bank early and
                # keeps both muls to a single PSUM operand each.
                t3 = work.tile([P, CH], f32, tag="t3", bufs=3)
                nc.vector.tensor_mul(t3, at, gt[:, qh, cs])
                return t3

            def norm_ag(qh, t3, bc):
                nc.vector.tensor_mul(ag[:, qh, :], t3, bc)

            def attn_tile(t, qh, at, dn, sc_name):
                o = t - c * ST
                off = o * P if o > 0 else 0  # live cols of diagonal tiles
                sc_ps = ps_sc.tile([P, CH], f32, tag="sc", name=sc_name)
                nc.tensor.matmul(
                    sc_ps[:, off:],
                    kro[:, t * P:(t + 1) * P],
                    qro[:, qh, c * CH + off:(c + 1) * CH],
                    start=True,
                    stop=True,
                )
                pr = work.tile([P, CH], bf16, tag="probs", bufs=4)
                nc.scalar.activation(
                    out=pr[:, off:], in_=sc_ps[:, off:], func=expf, scale=SCALE
                )
                if o >= 0:
                    nc.vector.tensor_mul(pr[:, off:], pr[:, off:], msk_sb[:, o, off:])
                return pr, off

            def attn_accum(t, pr, off, at, dn, dnst):
                nc.tensor.matmul(
                    at[:, off:], v_sb[:, t, :], pr[:, off:],
                    start=(t == 0), stop=(t == ntiles - 1),
                )
                # denominator: merge pairs of full-width tiles on the DVE so
                # one ones-matmul covers two probability tiles
                diag = t - c * ST >= 0
                if diag:
                    if dnst["pr"] is not None:
                        nc.tensor.matmul(
                            dn, ones_pv, dnst["pr"],
                            start=dnst["first"], stop=False,
                        )
                        dnst["pr"] = None
                        dnst["first"] = False
                    nc.tensor.matmul(
                        dn[:, off:], ones_pv, pr[:, off:],
                        start=dnst["first"], stop=(t == ntiles - 1),
                    )
                    dnst["first"] = False
                elif dnst["pr"] is None:
                    dnst["pr"] = pr
                else:
                    m = work.tile([P, CH], bf16, tag="dmrg", bufs=2)
                    nc.vector.tensor_add(m, dnst["pr"], pr)
                    nc.tensor.matmul(
                        dn, ones_pv, m, start=dnst["first"], stop=False,
                    )
                    dnst["pr"] = None
                    dnst["first"] = False

            pend = []  # [(qh, at, rc)] awaiting normalization
            for ha, hb in ((0, 1), (2, 3)):
                at_a = ps_pj.tile([P, CH], f32, tag="proj", name="at_a")
                at_b = ps_pj.tile([P, CH], f32, tag="proj", name="at_b")
                dn_a = ps_dn.tile([1, CH], f32, tag="dn", name="dn_a")
                dn_b = ps_dn.tile([1, CH], f32, tag="dn", name="dn_b")
                dnst_a = {"pr": None, "first": True}
                dnst_b = {"pr": None, "first": True}
                for t in range(ntiles):
                    pr_a, off = attn_tile(t, ha, at_a, dn_a, "sc_a")
                    pr_b, _ = attn_tile(t, hb, at_b, dn_b, "sc_b")
                    # normalize the previous pair inside the exp-latency
                    # bubble of this pair's first two tiles
                    if t < 2 and pend:
                        qh_p, at_p, rc_p = pend.pop(0)
                        t3_p = norm_t3(qh_p, at_p)
                        norm_ag(qh_p, t3_p, norm_bc(rc_p, f"bc{qh_p}"))
                    attn_accum(t, pr_a, off, at_a, dn_a, dnst_a)
                    attn_accum(t, pr_b, off, at_b, dn_b, dnst_b)
                rc_a = work.tile([1, CH], f32, tag="recip", bufs=4)
                nc.vector.reciprocal_approx_fast(out=rc_a, in_=dn_a)
                rc_b = work.tile([1, CH], f32, tag="recip", bufs=4)
                nc.vector.reciprocal_approx_fast(out=rc_b, in_=dn_b)
                pend += [(ha, at_a, rc_a), (hb, at_b, rc_b)]

            # o_proj of the previous chunk, with the last pair's
            # normalization woven into the first few groups.
            self_norm = pend
            pend = []
            if ag_prev is not None:
                emit_oproj(nc, ps_pj, outp, out, wo_sb, ag_prev, self_norm,
                           norm_bc, norm_t3, norm_ag)
            else:
                t3s = [norm_t3(qh, at) for qh, at, _ in self_norm]
                bcs = [norm_bc(rc, f"bc{qh}") for qh, _, rc in self_norm]
                for (qh, _, _), t3, bc in zip(self_norm, t3s, bcs):
                    norm_ag(qh, t3, bc)
            ag_prev = (c, ag)

        emit_oproj(nc, ps_pj, outp, out, wo_sb, ag_prev, [], None, None, None)

    nc.finalize()
    return nc


def emit_oproj(nc, ps_pj, outp, out, wo_sb, ag_info, norm2,
               norm_bc, norm_t3, norm_ag):
    f32 = mybir.dt.float32
    f16 = mybir.dt.float16
    c, ag = ag_info
    ST = CH // P
    bcs = []
    t3s = []
    gi = 0
    for st in range(ST):
        r0 = c * CH + st * P
        for h0 in range(HIDDEN // CH):
            if gi == 0 and norm2:
                # t3 muls free the at banks that groups 2/3 will reuse;
                # they only need the gate, so they run during group 0/1
                t3s = [norm_t3(qh, at) for qh, at, _ in norm2]
            if gi == 2 and norm2:
                # bc matmuls for the last pair (reciprocals are long done)
                bcs = [norm_bc(rc, f"bc{qh}") for qh, _, rc in norm2]
            if gi == 4 and norm2:
                for (qh, _, _), t3, bc in zip(norm2, t3s, bcs):
                    norm_ag(qh, t3, bc)
            ps = ps_pj.tile([P, CH], f32, tag="proj", name=f"op{gi}")
            for dt in range(QH):
                nc.tensor.matmul(
                    ps,
                    ag[:, dt, st * P:(st + 1) * P],
                    wo_sb[:, dt, h0 * CH:(h0 + 1) * CH],
                    start=(dt == 0),
                    stop=(dt == QH - 1),
                )
            ob = outp.tile([P, CH], f16, tag="ob")
            nc.vector.tensor_copy(out=ob, in_=ps)
            nc.sync.dma_start(out=out[r0:r0 + P, h0 * CH:(h0 + 1) * CH], in_=ob)
            gi += 1


_PROGRAMS = {}


def _get_program(S=S_FULL):
    if S not in _PROGRAMS:
        _PROGRAMS[S] = build_program(S)
    return _PROGRAMS[S]


def _host_tables(position_ids_b, S):
    pos = np.asarray(position_ids_b, dtype=np.float32)  # [S]
    inv = 1.0 / (ROPE_THETA ** (np.arange(0, P, 2, dtype=np.float32) / P))  # [64]
    ang = np.concatenate([inv, inv]).astype(np.float32)[:, None] * pos[None, :]
    cosT = np.cos(ang).astype(BF16)
    sgn = np.where(np.arange(P) < 64, -1.0, 1.0).astype(np.float32)
    sinT = (np.sin(ang) * sgn[:, None]).astype(BF16)
    return cosT, sinT


def _causal_masks():
    o = np.arange(CH // P)[:, None, None]
    r = np.arange(P)[None, :, None]
    j = np.arange(CH)[None, None, :]
    return ((P * o + r) <= j).astype(BF16)


def make_in_maps(x, position_ids, Wq, Wk, Wv, Wo, Wg, bg, S=S_FULL):
    x = np.asarray(x, dtype=np.float32)
    msk = _causal_masks()
    maps = []
    xT_b = [np.ascontiguousarray(x[b, :S].T).astype(BF16) for b in range(B)]
    tabs = [_host_tables(np.asarray(position_ids)[b, :S], S) for b in range(B)]
    Wq = np.asarray(Wq, np.float32)
    Wk = np.asarray(Wk, np.float32)
    Wv = np.asarray(Wv, np.float32)
    Wo = np.asarray(Wo, np.float32)
    Wg = np.asarray(Wg, np.float32)
    bg = np.asarray(bg, np.float32)
    for core in range(8):
        b, g = core // 4, core % 4
        cosT, sinT = tabs[b]
        maps.append({
            "xT": xT_b[b],
            "wq": np.ascontiguousarray(Wq[:, g * DQ:(g + 1) * DQ]).astype(BF16),
            "wk": np.ascontiguousarray(Wk[:, g * P:(g + 1) * P]).astype(BF16),
            "wv": np.ascontiguousarray(Wv[:, g * P:(g + 1) * P]).astype(BF16),
            "wg": np.ascontiguousarray(Wg[:, g * DQ:(g + 1) * DQ]).astype(BF16),
            "wo": np.ascontiguousarray(Wo[g * DQ:(g + 1) * DQ, :]).astype(BF16),
            "bg": np.ascontiguousarray(bg[g * DQ:(g + 1) * DQ]) * np.float32(0.5),
            "cosT": cosT,
            "sinT": sinT,
            "msk": msk,
        })
    return maps


def run(inputs, S=S_FULL, trace=False, **kw):
    nc = _get_program(S)
    maps = make_in_maps(S=S, **inputs)
    res = run_bass_kernel_spmd(nc, maps, core_ids=list(range(8)), trace=trace, **kw)
    out = np.zeros((B, S, HIDDEN), np.float32)
    for core in range(8):
        out[core // 4] += np.asarray(res.results[core]["out"], np.float32)
    return out, res


def kernel(x, position_ids, Wq, Wk, Wv, Wo, Wg, bg):
    out, _ = run(dict(x=x, position_ids=position_ids, Wq=Wq, Wk=Wk, Wv=Wv,
                      Wo=Wo, Wg=Wg, bg=bg))
    return out
